# revision 1
# baseline (speedup 1.0000x reference)
"""DeepMOI GNN kernel for 8 Trainium2 NeuronCores (Bass/Tile).

Sharding: stage-1 full-graph SAGE aggregation is sharded by dst-node slice
(1024 nodes/core) with an AllGather of the aggregated means; the dense
per-node compute is replicated. Stage-2 pathway subgraphs are data-parallel:
4 graphs per core, processed sequentially. readout2 rows are AllGathered
and the tiny final MLP is computed on every core.

Aggregation strategy: edges sorted by dst, packed into 128-dst windows;
segment-sum is a one-hot matmul on PE (one-hot built once on DVE from
static dst-local columns). Messages are bulk-gathered from a row table in
DRAM with dma_gather. Top-k masks use a 16-probe multisection search.
"""
import sys
sys.path.insert(0, '/opt/trn_rl_repo')
import numpy as np
import ml_dtypes

import concourse.bass as bass
import concourse.bacc as bacc
import concourse.tile as tile
import concourse.mybir as mybir
from concourse import bass_utils

N = 8000
NP = 8192
P = 32
ES = 4096
DIN = 16
D = 128
NC = 8
GPC = P // NC
W2 = 64
S1SLICE = NP // NC
S1W = S1SLICE // 128
KS = (6400, 5120, 4096)
NPROBE = 16
NITER = 4

AF = mybir.ActivationFunctionType
OP = mybir.AluOpType
dt = mybir.dt
BF = ml_dtypes.bfloat16
AX = mybir.AxisListType

_build_cache = {}


# ----------------------------------------------------------------- host prep
def wrap_idxs(idx):
    n = idx.shape[0]
    w = np.zeros((128, n // 16), dtype=np.int16)
    base = idx.reshape(n // 16, 16).T
    for k in range(8):
        w[16 * k:16 * (k + 1), :] = base
    return w


def window_major_edges(src, dst, nwin, cmax, sentinel):
    order = np.argsort(dst, kind='stable')
    src, dst = src[order], dst[order]
    nslot = nwin * cmax * 128
    sp = np.full(nslot, sentinel, np.int64)
    dl = np.full((128, nwin * cmax), -1.0, np.float32)
    for w in range(nwin):
        lo = np.searchsorted(dst, w * 128)
        hi = np.searchsorted(dst, (w + 1) * 128)
        if hi <= lo:
            continue
        assert hi - lo <= cmax * 128, f"window {w}: {hi-lo}"
        base = w * cmax * 128
        sp[base:base + (hi - lo)] = src[lo:hi]
        for k in range(cmax):
            a = lo + k * 128
            if a >= hi:
                break
            b = min(hi, a + 128)
            dl[:b - a, w * cmax + k] = (dst[a:b] - w * 128).astype(np.float32)
    return sp, dl


def host_prep(inputs):
    h = np.asarray(inputs['h'], np.float32)
    ei = np.asarray(inputs['edge_index'], np.int64)
    sei = np.asarray(inputs['sub_edge_index'], np.int64)

    src1 = np.concatenate([ei[0], np.arange(N)])
    dst1 = np.concatenate([ei[1], np.arange(N)])
    deg1 = np.bincount(dst1, minlength=NP).astype(np.float32)
    rdeg1 = 1.0 / np.maximum(deg1, 1.0)
    cnt_w = np.bincount(dst1 // 128, minlength=64)
    CMAX1 = int(np.ceil(cnt_w.max() / 128))

    s1 = []
    for c in range(NC):
        m = (dst1 >= c * S1SLICE) & (dst1 < (c + 1) * S1SLICE)
        sp, dl = window_major_edges(src1[m], dst1[m] - c * S1SLICE,
                                    S1W, CMAX1, NP - 1)
        npair = S1W * CMAX1
        re = np.zeros((128, npair), np.float32)
        for col in range(npair):
            w = col // CMAX1
            dloc = dl[:, col]
            dd = (c * S1SLICE + w * 128 + np.maximum(dloc, 0)).astype(np.int64)
            re[:, col] = np.where(dloc >= 0, rdeg1[dd], 0.0)
        s1.append((wrap_idxs(sp), dl, re))

    h_tab = np.zeros((NP, 128), BF)
    h_tab[:N, :DIN] = h.astype(BF)

    ssrc = (sei[0].reshape(P, ES) - (np.arange(P) * N)[:, None])
    sdst = (sei[1].reshape(P, ES) - (np.arange(P) * N)[:, None])
    CMAX2 = 1
    for g in range(P):
        CMAX2 = max(CMAX2, int(np.ceil(
            np.bincount(sdst[g] // 128, minlength=W2).max() / 128)))
    deg2 = np.zeros((P, NP), np.float32)
    for g in range(P):
        np.add.at(deg2[g], sdst[g], 1.0)
    rdeg2 = (1.0 / np.maximum(deg2, 1.0)).astype(BF)

    s2src, s2dl, s2rbc = [], [], []
    for c in range(NC):
        a_s, a_d, a_r = [], [], []
        for j in range(GPC):
            g = c * GPC + j
            sp, dl = window_major_edges(ssrc[g], sdst[g], W2, CMAX2, NP - 1)
            a_s.append(wrap_idxs(sp))
            a_d.append(dl)
            a_r.append(np.tile(rdeg2[g][None, :], (1, 1)))
        s2src.append(np.concatenate(a_s, axis=1))
        s2dl.append(np.concatenate(a_d, axis=1))
        s2rbc.append(np.stack([r[0] for r in a_r]))   # [GPC, NP] bf16

    iota128 = np.tile(np.arange(128, dtype=np.float32)[None, :], (128, 1))
    piota = np.arange(128, dtype=np.float32)[:, None]
    ident = (iota128 == piota).astype(BF)
    na0 = (np.arange(NP).reshape(W2, 128).T < N).astype(np.float32)
    probei = np.tile(np.arange(1, NPROBE + 1, dtype=np.float32)[None, :],
                     (1, 1))

    W = {k: np.asarray(inputs[k], np.float32) for k in inputs}
    roT1 = np.zeros((33, 1), np.float32)

    base = dict(
        h_tab=h_tab,
        iota128=iota128, ident=ident, identf=ident.astype(np.float32),
        na0=na0, probei=probei,
        Wl_a=W['Wl_a'].astype(BF), Wr_a=W['Wr_a'].astype(BF),
        Wl_s=np.ascontiguousarray(W['Wl_s'].transpose(1, 0, 2)).astype(BF),
        Wr_s=np.ascontiguousarray(W['Wr_s'].transpose(1, 0, 2)).astype(BF),
        bl_a=W['bl_a'][:, None], bl_s=np.ascontiguousarray(W['bl_s'].T),
        gate_w=np.ascontiguousarray(W['gate_w'].T).astype(BF),
        wrel=np.ascontiguousarray(W['pool_wrel'].T).astype(BF),
        wroot=np.ascontiguousarray(W['pool_wroot'].T).astype(BF),
        brel=np.tile(W['pool_brel'][None, :], (128, 1)),
        norm_w=W['norm_w'][:, None], norm_b=W['norm_b'][:, None],
        norm_ms=W['norm_ms'][:, None],
        lin_w=np.ascontiguousarray(W['lin_w'].reshape(3, 128).T),  # [128,3]
        lin_b33=np.full((33, 1), W['lin_b'][0], np.float32),
        m1_w=W['m1_w'], m1_b=W['m1_b'][:, None],
        m2_w=W['m2_w'], m2_b=W['m2_b'][:, None],
        m3_w=W['m3_w'], m3_b=W['m3_b'][:, None],
    )
    per_core = []
    for c in range(NC):
        m = dict(base)
        m['s1_src'], m['s1_dl'], m['s1_re'] = s1[c]
        m['s2_src'] = np.ascontiguousarray(s2src[c])
        m['s2_dl'] = np.ascontiguousarray(s2dl[c])
        m['s2_rbc0'] = np.ascontiguousarray(s2rbc[c])
        per_core.append(m)
    return per_core, (CMAX1, CMAX2)


# ---------------------------------------------------------------- the kernel
def build_nc(CMAX1, CMAX2, debug=False):
    NP1 = S1W * CMAX1 * 128
    NP2 = W2 * CMAX2 * 128
    PR1 = S1W * CMAX1
    PR2 = W2 * CMAX2

    nc = bacc.Bacc("TRN2", target_bir_lowering=False, debug=False,
                   num_devices=NC)
    f32, bf16 = dt.float32, dt.bfloat16

    def inp(name, shape, d=f32):
        return nc.dram_tensor(name, shape, d, kind="ExternalInput")

    t_htab = inp('h_tab', [NP, 128], bf16)
    t_s1src = inp('s1_src', [128, NP1 // 16], dt.int16)
    t_s1dl = inp('s1_dl', [128, PR1])
    t_s1re = inp('s1_re', [128, PR1])
    t_s2src = inp('s2_src', [128, GPC * NP2 // 16], dt.int16)
    t_s2dl = inp('s2_dl', [128, GPC * PR2])
    t_s2rbc = inp('s2_rbc0', [GPC, NP], bf16)
    t_iota = inp('iota128', [128, 128])
    t_ident = inp('ident', [128, 128], bf16)
    t_na0 = inp('na0', [128, W2])
    t_probei = inp('probei', [1, NPROBE])
    t_WlA = inp('Wl_a', [16, 128], bf16)
    t_WrA = inp('Wr_a', [16, 128], bf16)
    t_WlS = inp('Wl_s', [128, 5, 128], bf16)
    t_WrS = inp('Wr_s', [128, 5, 128], bf16)
    t_blA = inp('bl_a', [128, 1])
    t_blS = inp('bl_s', [128, 5])
    t_gate = inp('gate_w', [128, 6], bf16)
    t_wrel = inp('wrel', [128, 3], bf16)
    t_wroot = inp('wroot', [128, 3], bf16)
    t_brel = inp('brel', [128, 3])
    t_identf = inp('identf', [128, 128])
    t_nw = inp('norm_w', [128, 1])
    t_nb = inp('norm_b', [128, 1])
    t_nms = inp('norm_ms', [128, 1])
    t_linw = inp('lin_w', [128, 3])
    t_linb = inp('lin_b33', [33, 1])
    t_m1w = inp('m1_w', [33, 48])
    t_m1b = inp('m1_b', [48, 1])
    t_m2w = inp('m2_w', [48, 16])
    t_m2b = inp('m2_b', [16, 1])
    t_m3w = inp('m3_w', [16, 1])
    t_m3b = inp('m3_b', [1, 1])

    out = nc.dram_tensor('out', [1, 1], f32, kind="ExternalOutput")
    dbg = {}
    if debug:
        for nm, shp in (('d_x1T', [128, NP]), ('d_ro', [33, 384]),
                        ('d_sc0', [128, W2]), ('d_na0', [128, W2])):
            dbg[nm] = nc.dram_tensor(nm, shp, f32, kind="ExternalOutput")

    with tile.TileContext(nc, trace_sim=False) as tc:
        cp = tc.alloc_tile_pool(name="const", bufs=1)
        big = tc.alloc_tile_pool(name="big", bufs=1)
        wp = tc.alloc_tile_pool(name="wk", bufs=2)
        pp = tc.alloc_tile_pool(name="ps", bufs=2, space="PSUM")
        dp = tc.alloc_tile_pool(name="dram", bufs=1, space="DRAM")

        # ------------- constants
        def load(tt, shape, d=f32, pool=cp):
            s = pool.tile(shape, d, tag=f"ld_{tt.name}")
            nc.sync.dma_start(s[:], tt.ap())
            return s

        iota = load(t_iota, [128, 128])
        ident = load(t_ident, [128, 128], bf16)
        na0c = load(t_na0, [128, W2])
        probei = load(t_probei, [1, NPROBE])
        WlA = load(t_WlA, [16, 128], bf16)
        WrA = load(t_WrA, [16, 128], bf16)
        WlS = load(t_WlS, [128, 5, 128], bf16)
        WrS = load(t_WrS, [128, 5, 128], bf16)
        blA = load(t_blA, [128, 1])
        blS = load(t_blS, [128, 5])
        gate = load(t_gate, [128, 6], bf16)
        wrel = load(t_wrel, [128, 3], bf16)
        wroot = load(t_wroot, [128, 3], bf16)
        brel = load(t_brel, [128, 3])
        identf = load(t_identf, [128, 128])
        nw = load(t_nw, [128, 1]); nb = load(t_nb, [128, 1])
        nms = load(t_nms, [128, 1])
        linw = load(t_linw, [128, 3])
        linb = load(t_linb, [33, 1])
        m1w = load(t_m1w, [33, 48]); m1b = load(t_m1b, [48, 1])
        m2w = load(t_m2w, [48, 16]); m2b = load(t_m2b, [16, 1])
        m3w = load(t_m3w, [16, 1]); m3b = load(t_m3b, [1, 1])
        ones_c = cp.tile([128, 1], f32); nc.vector.memset(ones_c[:], 1.0)
        ones_r = cp.tile([1, 128], f32); nc.vector.memset(ones_r[:], 1.0)
        ones_rb = cp.tile([1, 128], bf16); nc.vector.memset(ones_rb[:], 1.0)
        ninf = cp.tile([128, W2], f32); nc.vector.memset(ninf[:], -1e30)

        xT = big.tile([128, NP], bf16, tag="xT")       # current features^T
        rows = big.tile([128, W2, 128], bf16, tag="rows")
        ro2s = cp.tile([128, 3, GPC], f32)
        ro1s = cp.tile([128, 3], f32)
        xrows = dp.tile([NP, 128], bf16)               # row table (DRAM)
        utab = dp.tile([NP, 128], bf16)                # u table (DRAM)

        # =========================================================== helpers
        def transpose_pass(srcT, also_dram):
            """srcT [128, NP] bf16 -> rows [128, W2, 128]; optionally DMA to
            xrows."""
            for w in range(W2):
                tp = pp.tile([128, 128], bf16, space="PSUM", tag="psA")
                nc.tensor.transpose(tp[:], srcT[:, w * 128:(w + 1) * 128],
                                    ident[:])
                nc.scalar.copy(out=rows[:, w, :], in_=tp[:])
            if also_dram:
                nc.sync.dma_start(
                    xrows[:].rearrange("(w p) e -> p w e", p=128), rows[:])

        rowbounce = dp.tile([1, NP], bf16)

        def row_from_grid(gtile, rowt):
            nc.gpsimd.dma_start(
                rowbounce[0:1, :].rearrange("o (w p) -> o p w", p=128),
                gtile[:])
            nc.sync.dma_start(rowt[:], rowbounce[:])

        def pe_bcast_col(val11):
            """[1,1] f32 -> psum [128,1]."""
            ps = pp.tile([128, 1], f32, space="PSUM", tag="psB")
            nc.tensor.matmul(ps[:], lhsT=ones_r[:], rhs=val11[:],
                             start=True, stop=True)
            return ps

        def readout(xmT, gate_i, na_g, f_g, slot, use_mask):
            """global-attention readout -> slot [128,1].
            a = softmax over alive of (f * (gate.x)); r = sum a*xm via rows."""
            gx = pp.tile([128, W2], f32, space="PSUM", tag="psB")
            for w in range(W2):
                nc.tensor.matmul(gx[:, w:w + 1],
                                 lhsT=xmT[:, w * 128:(w + 1) * 128],
                                 rhs=gate[:, gate_i:gate_i + 1],
                                 start=True, stop=True)
            gxs = wp.tile([128, W2], f32, tag="gxs")
            if use_mask:
                nc.vector.tensor_tensor(out=gxs[:], in0=gx[:], in1=f_g[:],
                                        op=OP.mult)
            else:
                nc.vector.tensor_copy(out=gxs[:], in_=gx[:])
            ex = wp.tile([128, W2], f32, tag="ex")
            nc.scalar.activation(ex[:], gxs[:], AF.Exp)
            nc.vector.tensor_tensor(out=ex[:], in0=ex[:],
                                    in1=(na_g[:] if use_mask else na0c[:]),
                                    op=OP.mult)
            sm = wp.tile([128, 1], f32, tag="sm")
            nc.vector.tensor_reduce(out=sm[:], in_=ex[:], axis=AX.X, op=OP.add)
            tot = pp.tile([1, 1], f32, space="PSUM", tag="psB")
            nc.tensor.matmul(tot[:], lhsT=sm[:], rhs=ones_c[:],
                             start=True, stop=True)
            rden = wp.tile([1, 1], f32, tag="rden")
            nc.vector.reciprocal(rden[:], tot[:])
            rdb = pe_bcast_col(rden)
            ab = wp.tile([128, W2], bf16, tag="ab")
            nc.vector.tensor_scalar(out=ab[:], in0=ex[:], scalar1=rdb[:, 0:1],
                                    scalar2=None, op0=OP.mult)
            rps = pp.tile([128, 1], f32, space="PSUM", tag="psB")
            for w in range(W2):
                nc.tensor.matmul(rps[:], lhsT=rows[:, w, :],
                                 rhs=ab[:, w:w + 1],
                                 start=(w == 0), stop=(w == W2 - 1))
            nc.vector.tensor_copy(out=slot, in_=rps[:])

        # ====================================================== stage 1
        s1pool = tc.alloc_tile_pool(name="s1", bufs=1)
        s1src = load(t_s1src, [128, NP1 // 16], dt.int16, pool=s1pool)
        s1dl = load(t_s1dl, [128, PR1], pool=s1pool)
        s1re = load(t_s1re, [128, PR1], pool=s1pool)
        eq1 = s1pool.tile([128, PR1, 128], bf16)
        for pr in range(PR1):
            ef = wp.tile([128, 128], f32, tag="ef")
            nc.vector.tensor_scalar(out=ef[:], in0=iota[:],
                                    scalar1=s1dl[:, pr:pr + 1], scalar2=None,
                                    op0=OP.is_equal)
            nc.vector.tensor_scalar(out=eq1[:, pr, :], in0=ef[:],
                                    scalar1=s1re[:, pr:pr + 1], scalar2=None,
                                    op0=OP.mult)

        nc.sync.dma_start_transpose(xT[:], t_htab.ap())

        agb_in = dp.tile([128, S1SLICE], bf16)
        agb_out = dp.tile([128 * NC, S1SLICE], bf16)
        meanT = big.tile([128, NP], bf16, tag="meanT")

        def s1_layer(layer):
            srcT = xT
            src_tab = t_htab.ap() if layer == 0 else xrows[:]
            aggp = pp.tile([128, S1W, 128], f32, space="PSUM", tag="pagg")
            for half in range(2):
                msgs = s1pool.tile([128, PR1 // 2, 128], bf16, tag="msgs1")
                nc.gpsimd.dma_gather(
                    out_ap=msgs[:], in_ap=src_tab,
                    idxs_ap=s1src[:, half * (NP1 // 32):(half + 1) * (NP1 // 32)],
                    num_idxs=NP1 // 2, num_idxs_reg=NP1 // 2, elem_size=128,
                    single_packet=False)
                for w in range(S1W // 2 * half, S1W // 2 * (half + 1)):
                    for k in range(CMAX1):
                        pr = w * CMAX1 + k
                        lpr = pr - half * (PR1 // 2)
                        nc.tensor.matmul(aggp[:, w, :], lhsT=msgs[:, lpr, :],
                                         rhs=eq1[:, pr, :], start=(k == 0),
                                         stop=(k == CMAX1 - 1))
            mslice = wp.tile([128, S1SLICE], bf16, tag="mslice")
            nc.scalar.copy(out=mslice[:],
                           in_=aggp[:].rearrange("p w e -> p (w e)"))
            nc.sync.dma_start(agb_in[:], mslice[:])
            nc.gpsimd.collective_compute(
                "AllGather", OP.bypass, replica_groups=[list(range(NC))],
                ins=[agb_in.opt()], outs=[agb_out.opt()])
            # reassemble meanT full [128, NP]
            nc.sync.dma_start(
                meanT[:].rearrange("p (c s) -> p c s", c=NC),
                agb_out[:].rearrange("(c p) s -> p c s", p=128))
            Wl = WlA if layer == 0 else WlS[:, layer - 1, :]
            Wr = WrA if layer == 0 else WrS[:, layer - 1, :]
            bl = blA if layer == 0 else blS[:, layer - 1:layer]
            kdim = 16 if layer == 0 else 128
            for t in range(NP // 512):
                sl = slice(t * 512, (t + 1) * 512)
                xp = pp.tile([128, 512], f32, space="PSUM", tag="psA")
                nc.tensor.matmul(xp[:], lhsT=Wl[0:kdim, :],
                                 rhs=meanT[0:kdim, sl], start=True, stop=False)
                nc.tensor.matmul(xp[:], lhsT=Wr[0:kdim, :],
                                 rhs=srcT[0:kdim, sl], start=False, stop=True)
                nc.scalar.activation(xT[:, sl], xp[:], AF.Tanh, bias=bl[:])

        def gnorm():
            mu = wp.tile([128, 1], f32, tag="mu")
            nc.vector.tensor_reduce(out=mu[:], in_=xT[:, 0:N], axis=AX.X,
                                    op=OP.add)
            nc.vector.tensor_scalar(out=mu[:], in0=mu[:], scalar1=1.0 / N,
                                    scalar2=None, op0=OP.mult)
            mums = wp.tile([128, 1], f32, tag="mums")
            nc.vector.tensor_tensor(out=mums[:], in0=mu[:], in1=nms[:],
                                    op=OP.mult)
            o = s1pool.tile([128, NP], bf16, tag="onorm")
            nc.vector.tensor_scalar(out=o[:], in0=xT[:], scalar1=mums[:],
                                    scalar2=None, op0=OP.subtract)
            var = wp.tile([128, 1], f32, tag="var")
            sq = s1pool.tile([128, NP], bf16, tag="sqnorm")
            nc.vector.tensor_tensor(out=sq[:], in0=o[:], in1=o[:], op=OP.mult)
            nc.vector.tensor_reduce(out=var[:], in_=sq[:, 0:N], axis=AX.X,
                                    op=OP.add)
            nc.vector.tensor_scalar(out=var[:], in0=var[:], scalar1=1.0 / N,
                                    scalar2=1e-5, op0=OP.mult, op1=OP.add)
            rstd = wp.tile([128, 1], f32, tag="rstd")
            nc.vector.reciprocal(rstd[:], var[:])
            nc.scalar.activation(rstd[:], rstd[:], AF.Sqrt)
            sc = wp.tile([128, 1], f32, tag="scn")
            nc.vector.tensor_tensor(out=sc[:], in0=rstd[:], in1=nw[:],
                                    op=OP.mult)
            nc.vector.tensor_scalar(out=xT[:], in0=o[:], scalar1=sc[:],
                                    scalar2=nb[:], op0=OP.mult, op1=OP.add)

        for layer in range(3):
            s1_layer(layer)
            transpose_pass(xT, also_dram=False)
            readout(xT, layer, None, None, ro1s[:, layer:layer + 1],
                    use_mask=False)
            if layer < 2:
                gnorm()
            transpose_pass(xT, also_dram=True)

        if dbg:
            nc.gpsimd.dma_start(dbg['d_x1T'].ap(), xT[:])

        s1pool.release()

        # ====================================================== stage 2
        s2pool = tc.alloc_tile_pool(name="s2", bufs=1)
        s2g = tc.alloc_tile_pool(name="s2g", bufs=2)
        s2src = s2pool.tile([128, GPC * NP2 // 16], dt.int16, tag="s2src")
        nc.sync.dma_start(s2src[:], t_s2src.ap())
        s2dl = s2pool.tile([128, GPC * PR2], f32, tag="s2dl")
        nc.sync.dma_start(s2dl[:], t_s2dl.ap())

        xgT = s2pool.tile([128, NP], bf16, tag="xgT")
        eq2 = s2pool.tile([128, PR2, 128], bf16, tag="eq2")
        rbc = s2pool.tile([128, NP], bf16, tag="rbc")
        frow = cp.tile([1, NP], bf16)

        for j in range(GPC):
            # per-graph eq tiles (unscaled one-hot)
            for pr in range(PR2):
                nc.vector.tensor_scalar(
                    out=eq2[:, pr, :], in0=iota[:],
                    scalar1=s2dl[:, j * PR2 + pr:j * PR2 + pr + 1],
                    scalar2=None, op0=OP.is_equal)
            nc.vector.tensor_copy(out=xgT[:], in_=xT[:])
            na_g = cp.tile([128, W2], f32, tag=f"nag{j}")
            f_g = cp.tile([128, W2], f32, tag=f"fg{j}")
            nc.vector.tensor_copy(out=na_g[:], in_=na0c[:])

            for l in range(3):
                li = 2 + l
                # ---- messages gather (from stage-1 table at l=0, else own)
                msgs = s2g.tile([128, PR2, 128], bf16, tag="m2")
                nc.gpsimd.dma_gather(
                    out_ap=msgs[:], in_ap=xrows[:],
                    idxs_ap=s2src[:, j * (NP2 // 16):(j + 1) * (NP2 // 16)],
                    num_idxs=NP2, num_idxs_reg=NP2, elem_size=128,
                    single_packet=False)
                # ---- rbc (mean scale broadcast, bf16 [128, NP])
                if l == 0:
                    nc.sync.dma_start(frow[:], t_s2rbc.ap()[j:j + 1, :])
                else:
                    # deg from indicator trick + eq matmuls
                    ind = wp.tile([128, PR2], bf16, tag="ind")
                    nc.vector.tensor_scalar(out=ind[:],
                                            in0=msgs[:, :, 0:1].rearrange(
                                                "p c o -> p (c o)"),
                                            scalar1=0.0, scalar2=None,
                                            op0=OP.not_equal)
                    degp = pp.tile([128, W2], f32, space="PSUM", tag="psB")
                    for w in range(W2):
                        for k in range(CMAX2):
                            pr = w * CMAX2 + k
                            nc.tensor.matmul(
                                degp[:, w:w + 1], lhsT=eq2[:, pr, :],
                                rhs=ind[:, pr:pr + 1], start=(k == 0),
                                stop=(k == CMAX2 - 1))
                    rdeg = wp.tile([128, W2], f32, tag="rdeg")
                    nc.vector.tensor_scalar(out=rdeg[:], in0=degp[:],
                                            scalar1=1.0, scalar2=None,
                                            op0=OP.max)
                    nc.vector.reciprocal(rdeg[:], rdeg[:])
                    nc.vector.tensor_tensor(out=rdeg[:], in0=rdeg[:],
                                            in1=na_g[:], op=OP.mult)
                    row_from_grid(rdeg, frow)
                for t in range(NP // 512):
                    sl = slice(t * 512, (t + 1) * 512)
                    ob = pp.tile([128, 512], f32, space="PSUM", tag="psA")
                    nc.tensor.matmul(ob[:], lhsT=ones_rb[:],
                                     rhs=frow[0:1, sl], start=True,
                                     stop=True)
                    nc.scalar.copy(out=rbc[:, sl], in_=ob[:])
                # ---- aggregation matmuls + fused mean scale
                for grp in range(8):
                    agp = pp.tile([128, 8, 128], f32, space="PSUM",
                                  tag="pagg")
                    for wi in range(8):
                        w = grp * 8 + wi
                        for k in range(CMAX2):
                            pr = w * CMAX2 + k
                            nc.tensor.matmul(agp[:, wi, :],
                                             lhsT=msgs[:, pr, :],
                                             rhs=eq2[:, pr, :],
                                             start=(k == 0),
                                             stop=(k == CMAX2 - 1))
                    sl = slice(grp * 1024, (grp + 1) * 1024)
                    nc.vector.tensor_tensor(
                        out=meanT[:, sl],
                        in0=agp[:].rearrange("p w e -> p (w e)"),
                        in1=rbc[:, sl], op=OP.mult)
                # ---- x' = tanh(Wl.T meanT + Wr.T xm + bl)
                for t in range(NP // 512):
                    sl = slice(t * 512, (t + 1) * 512)
                    xp = pp.tile([128, 512], f32, space="PSUM", tag="psA")
                    nc.tensor.matmul(xp[:], lhsT=WlS[:, li, :], rhs=meanT[:, sl],
                                     start=True, stop=False)
                    nc.tensor.matmul(xp[:], lhsT=WrS[:, li, :], rhs=xgT[:, sl],
                                     start=False, stop=True)
                    nc.scalar.activation(xgT[:, sl], xp[:], AF.Tanh,
                                         bias=blS[:, li:li + 1])
                # ---- u/uroot/gx grids
                ug = pp.tile([128, W2, 3], f32, space="PSUM", tag="psB")
                wcols = cp.tile([128, 3], bf16, tag=f"wcols{j}_{l}")
                nc.vector.tensor_copy(out=wcols[:, 0:1], in_=wrel[:, l:l + 1])
                nc.vector.tensor_copy(out=wcols[:, 1:2],
                                      in_=wroot[:, l:l + 1])
                nc.vector.tensor_copy(out=wcols[:, 2:3],
                                      in_=gate[:, 3 + l:4 + l])
                for w in range(W2):
                    nc.tensor.matmul(ug[:, w, :],
                                     lhsT=xgT[:, w * 128:(w + 1) * 128],
                                     rhs=wcols[:], start=True, stop=True)
                ugs = wp.tile([128, W2, 3], f32, tag="ugs")
                nc.vector.tensor_copy(out=ugs[:], in_=ug[:])
                # write u table (strided, cast bf16)
                nc.gpsimd.dma_start(
                    utab[:].rearrange("(w p) e -> p w e", p=128)[:, :, 0:1],
                    ugs[:, :, 0:1])
                # ---- score pass
                umsg = s2g.tile([128, PR2, 128], bf16, tag="m2")
                nc.gpsimd.dma_gather(
                    out_ap=umsg[:], in_ap=utab[:],
                    idxs_ap=s2src[:, j * (NP2 // 16):(j + 1) * (NP2 // 16)],
                    num_idxs=NP2, num_idxs_reg=NP2, elem_size=128,
                    single_packet=False)
                scp = pp.tile([128, W2], f32, space="PSUM", tag="psB")
                for w in range(W2):
                    for k in range(CMAX2):
                        pr = w * CMAX2 + k
                        nc.tensor.matmul(scp[:, w:w + 1], lhsT=eq2[:, pr, :],
                                         rhs=umsg[:, pr, 0:1],
                                         start=(k == 0),
                                         stop=(k == CMAX2 - 1))
                score = cp.tile([128, W2], f32, tag=f"score{j}_{l}")
                nc.vector.tensor_tensor(out=score[:], in0=scp[:],
                                        in1=ugs[:, :, 1], op=OP.add)
                nc.vector.tensor_scalar(out=score[:], in0=score[:],
                                        scalar1=brel[:, l:l + 1],
                                        scalar2=None, op0=OP.add)
                if dbg and j == 0 and l == 0:
                    nc.gpsimd.dma_start(dbg['d_sc0'].ap(), score[:])
                # ---- top-k threshold via multiprobe
                sm_ = cp.tile([128, W2], f32, tag=f"smask{j}_{l}")
                nc.vector.tensor_tensor(out=sm_[:], in0=score[:], in1=na_g[:],
                                        op=OP.mult)
                pen = wp.tile([128, W2], f32, tag="pen")
                nc.vector.tensor_scalar(out=pen[:], in0=na_g[:],
                                        scalar1=1e6, scalar2=-1e6,
                                        op0=OP.mult, op1=OP.add)
                nc.vector.tensor_tensor(out=sm_[:], in0=sm_[:], in1=pen[:],
                                        op=OP.add)
                lo = cp.tile([1, 1], f32, tag=f"lo{j}_{l}")
                st = cp.tile([1, 1], f32, tag=f"st{j}_{l}")
                nc.vector.memset(lo[:], -16.0)
                nc.vector.memset(st[:], 32.0 / NPROBE)
                kk = float(KS[l])  # per-graph keep count
                for it in range(NITER):
                    pr_ = wp.tile([1, NPROBE], f32, tag="pr_")
                    nc.vector.tensor_scalar(out=pr_[:], in0=probei[:],
                                            scalar1=st[:, 0:1], scalar2=None,
                                            op0=OP.mult)
                    nc.vector.tensor_scalar(out=pr_[:], in0=pr_[:],
                                            scalar1=lo[:, 0:1], scalar2=None,
                                            op0=OP.add)
                    pb = pp.tile([128, NPROBE], f32, space="PSUM", tag="psB")
                    nc.tensor.matmul(pb[:], lhsT=ones_r[:], rhs=pr_[:],
                                     start=True, stop=True)
                    cmp_ = wp.tile([128, NPROBE, W2], f32, tag="cmp_")
                    nc.vector.tensor_tensor(
                        out=cmp_[:],
                        in0=sm_[:].rearrange("p (o w) -> p o w", o=1)
                            .to_broadcast([128, NPROBE, W2]),
                        in1=pb[:].rearrange("p (r o) -> p r o", o=1)
                            .to_broadcast([128, NPROBE, W2]),
                        op=OP.is_ge)
                    cnt = wp.tile([128, NPROBE], f32, tag="cnt")
                    nc.vector.tensor_reduce(out=cnt[:], in_=cmp_[:],
                                            axis=AX.X, op=OP.add)
                    cs = pp.tile([1, NPROBE], f32, space="PSUM", tag="psB")
                    nc.tensor.matmul(cs[:], lhsT=ones_c[:], rhs=cnt[:],
                                     start=True, stop=True)
                    sges = wp.tile([1, NPROBE], f32, tag="sges")
                    nc.vector.tensor_scalar(out=sges[:], in0=cs[:],
                                            scalar1=kk - 0.5, scalar2=None,
                                            op0=OP.is_ge)
                    s8 = wp.tile([1, 1], f32, tag="s8")
                    nc.vector.tensor_reduce(out=s8[:], in_=sges[:],
                                            axis=AX.X, op=OP.add)
                    nc.vector.tensor_tensor(out=s8[:], in0=s8[:],
                                            in1=st[:], op=OP.mult)
                    nc.vector.tensor_tensor(out=lo[:], in0=lo[:], in1=s8[:],
                                            op=OP.add)
                    nc.vector.tensor_scalar(out=st[:], in0=st[:],
                                            scalar1=1.0 / NPROBE,
                                            scalar2=None, op0=OP.mult)
                thr = wp.tile([1, 1], f32, tag="thr")
                nc.vector.tensor_scalar(out=thr[:], in0=st[:],
                                        scalar1=float(NPROBE) / 2,
                                        scalar2=None, op0=OP.mult)
                nc.vector.tensor_tensor(out=thr[:], in0=thr[:], in1=lo[:],
                                        op=OP.add)
                thb = pe_bcast_col(thr)
                nc.vector.tensor_scalar(out=na_g[:], in0=sm_[:],
                                        scalar1=thb[:, 0:1], scalar2=None,
                                        op0=OP.is_ge)
                if dbg and j == 0 and l == 0:
                    nc.gpsimd.dma_start(dbg['d_na0'].ap(), na_g[:])
                # f = na * tanh(score)
                nc.scalar.activation(f_g[:], score[:], AF.Tanh)
                nc.vector.tensor_tensor(out=f_g[:], in0=f_g[:], in1=na_g[:],
                                        op=OP.mult)
                # ---- mask xgT in place: xm = x * f  (column scale).
                # At l==2 nothing downstream needs xmT densely (readout
                # folds f into the attention weights), so skip the scale.
                if l < 2:
                    row_from_grid(f_g, frow)
                    for t in range(NP // 512):
                        sl = slice(t * 512, (t + 1) * 512)
                        ob = pp.tile([128, 512], f32, space="PSUM", tag="psA")
                        nc.tensor.matmul(ob[:], lhsT=ones_rb[:],
                                         rhs=frow[0:1, sl], start=True,
                                         stop=True)
                        nc.vector.tensor_tensor(out=xgT[:, sl],
                                                in0=xgT[:, sl],
                                                in1=ob[:], op=OP.mult)
                # ---- rows of xm (+ DMA for next layer's gather)
                transpose_pass(xgT, also_dram=(l < 2))
                # ---- readout (gx grid = ug[:,:,2] is pre-mask -> fold f)
                gxg = wp.tile([128, W2], f32, tag="gxg")
                nc.vector.tensor_tensor(out=gxg[:], in0=ugs[:, :, 2],
                                        in1=f_g[:], op=OP.mult)
                ex = wp.tile([128, W2], f32, tag="ex")
                nc.scalar.activation(ex[:], gxg[:], AF.Exp)
                nc.vector.tensor_tensor(out=ex[:], in0=ex[:], in1=na_g[:],
                                        op=OP.mult)
                smr = wp.tile([128, 1], f32, tag="smr")
                nc.vector.tensor_reduce(out=smr[:], in_=ex[:], axis=AX.X,
                                        op=OP.add)
                tot = pp.tile([1, 1], f32, space="PSUM", tag="psB")
                nc.tensor.matmul(tot[:], lhsT=smr[:], rhs=ones_c[:],
                                 start=True, stop=True)
                rden = wp.tile([1, 1], f32, tag="rden")
                nc.vector.reciprocal(rden[:], tot[:])
                rdb = pe_bcast_col(rden)
                ab = wp.tile([128, W2], bf16, tag="ab")
                nc.vector.tensor_scalar(out=ab[:], in0=ex[:],
                                        scalar1=rdb[:, 0:1], scalar2=None,
                                        op0=OP.mult)
                if l == 2:
                    nc.vector.tensor_tensor(out=ab[:], in0=ab[:],
                                            in1=f_g[:], op=OP.mult)
                rps = pp.tile([128, 1], f32, space="PSUM", tag="psB")
                for w in range(W2):
                    nc.tensor.matmul(rps[:], lhsT=rows[:, w, :],
                                     rhs=ab[:, w:w + 1], start=(w == 0),
                                     stop=(w == W2 - 1))
                nc.vector.tensor_copy(out=ro2s[:, l, j:j + 1], in_=rps[:])

        # ====================================================== final MLP
        rob_in = dp.tile([GPC, 3 * 128], f32)
        rob_out = dp.tile([P, 3 * 128], f32)
        for j in range(GPC):
            nc.sync.dma_start(
                rob_in[j:j + 1, :].rearrange("g (l p) -> p (g l)", p=128),
                ro2s[:, :, j])
        nc.gpsimd.collective_compute(
            "AllGather", OP.bypass, replica_groups=[list(range(NC))],
            ins=[rob_in.opt()], outs=[rob_out.opt()])
        roall = cp.tile([32, 384], f32)
        nc.sync.dma_start(roall[:], rob_out[:])
        roT = cp.tile([128, 3, 33], f32)
        nc.vector.tensor_copy(out=roT[:, :, 0:1],
                              in_=ro1s[:].rearrange("p (l o) -> p l o", o=1))
        for lblk in range(3):
            tp = pp.tile([128, 32], f32, space="PSUM", tag="psB")
            nc.tensor.transpose(tp[:, 0:32],
                                roall[:, lblk * 128:(lblk + 1) * 128],
                                identf[0:32, 0:32])
            nc.vector.tensor_copy(out=roT[:, lblk, 1:33], in_=tp[:, 0:32])
        zp = pp.tile([33, 1], f32, space="PSUM", tag="psB")
        for lblk in range(3):
            nc.tensor.matmul(zp[:], lhsT=roT[:, lblk, :],
                             rhs=linw[:, lblk:lblk + 1], start=(lblk == 0),
                             stop=(lblk == 2))
        z = cp.tile([33, 1], f32)
        nc.scalar.activation(z[:], zp[:], AF.Tanh, bias=linb[:])
        h1p = pp.tile([48, 1], f32, space="PSUM", tag="psB")
        nc.tensor.matmul(h1p[:], lhsT=m1w[:], rhs=z[:], start=True, stop=True)
        h1 = cp.tile([48, 1], f32)
        nc.scalar.activation(h1[:], h1p[:], AF.Tanh, bias=m1b[:])
        h2p = pp.tile([16, 1], f32, space="PSUM", tag="psB")
        nc.tensor.matmul(h2p[:], lhsT=m2w[:], rhs=h1[:], start=True,
                         stop=True)
        h2 = cp.tile([16, 1], f32)
        nc.scalar.activation(h2[:], h2p[:], AF.Tanh, bias=m2b[:])
        h3p = pp.tile([1, 1], f32, space="PSUM", tag="psB")
        nc.tensor.matmul(h3p[:], lhsT=m3w[:], rhs=h2[:], start=True,
                         stop=True)
        h3 = cp.tile([1, 1], f32)
        nc.scalar.activation(h3[:], h3p[:], AF.Sigmoid, bias=m3b[:])
        nc.scalar.activation(h3[:], h3[:], AF.Sigmoid)
        nc.sync.dma_start(out.ap(), h3[:])
        if dbg:
            nc.gpsimd.dma_start(dbg['d_ro'].ap()[1:33, :], roall[:])
            nc.gpsimd.dma_start(
                dbg['d_ro'].ap()[0:1, :].rearrange("o (l p) -> p (o l)",
                                                   p=128), ro1s[:])

        for pool in (s2g, s2pool, dp, pp, wp, big, cp):
            pool.release()

    nc.compile()
    return nc


# ------------------------------------------------------------------- driver
def kernel(**inputs):
    per_core, meta = host_prep(inputs)
    key = meta
    if key not in _build_cache:
        _build_cache[key] = build_nc(*meta, debug=bool(
            int(__import__('os').environ.get('DMOI_DEBUG', '0'))))
    nc = _build_cache[key]
    import os as _os
    want_trace = bool(int(_os.environ.get('DMOI_TRACE', '0')))
    try:
        res = bass_utils.run_bass_kernel_spmd(
            nc, per_core, core_ids=list(range(NC)), trace=want_trace)
    except Exception:
        if not want_trace:
            raise
        res = bass_utils.run_bass_kernel_spmd(
            nc, per_core, core_ids=list(range(NC)))
    r0 = res.results[0]
    kernel.last_results = res
    return r0['out'].astype(np.float32)



# revision 3
# speedup vs baseline: 3.7660x; 3.7660x over previous
"""DeepMOI GNN kernel for 8 Trainium2 NeuronCores (Bass/Tile).

Sharding: stage-1 full-graph SAGE aggregation is sharded by dst-node slice
(1024 nodes/core) with an AllGather of the aggregated means; the dense
per-node compute is replicated. Stage-2 pathway subgraphs are data-parallel:
4 graphs per core, processed sequentially. readout2 rows are AllGathered
and the tiny final MLP is computed on every core.

Aggregation strategy: edges sorted by dst, packed into 128-dst windows;
segment-sum is a one-hot matmul on PE (one-hot built once on DVE from
static dst-local columns). Messages are bulk-gathered from a row table in
DRAM with dma_gather. Top-k masks use a 16-probe multisection search.

Host->device traffic is minimized: h ships transposed/packed ([16,8192]
bf16), gather indices ship unreplicated ([16,X] int16, fanned out to 128
partitions on device), reciprocal degrees are computed on device from the
one-hot tiles, iota/identity/window masks are generated on device, and the
small weights ride in three packed blobs. The jax persistent compilation
cache is enabled so repeat launches skip XLA re-compilation.
"""
import sys
sys.path.insert(0, '/opt/trn_rl_repo')
import numpy as np
import ml_dtypes

import jax
try:
    jax.config.update("jax_compilation_cache_dir", "/tmp/.dmoi_jax_cache")
    jax.config.update("jax_persistent_cache_min_entry_size_bytes", -1)
    jax.config.update("jax_persistent_cache_min_compile_time_secs", 0)
except Exception:
    pass

import concourse.bass as bass
import concourse.bacc as bacc
import concourse.tile as tile
import concourse.mybir as mybir
from concourse import bass_utils

N = 8000
NP = 8192
P = 32
ES = 4096
DIN = 16
D = 128
NC = 8
GPC = P // NC
W2 = 64
S1SLICE = NP // NC
S1W = S1SLICE // 128
KS = (6400, 5120, 4096)
NPROBE = 16
NITER = 4

AF = mybir.ActivationFunctionType
OP = mybir.AluOpType
dt = mybir.dt
BF = ml_dtypes.bfloat16
AX = mybir.AxisListType

_build_cache = {}

# Fb blob column map (f32 [128, FBW])
FB_BLA = 0          # [128, 1]
FB_BLS = 1          # [128, 5]
FB_BREL = 6         # [128, 3]
FB_NW = 9
FB_NB = 10
FB_NMS = 11
FB_LINW = 12        # [128, 3]
FB_M1W = 15         # [33, 48]
FB_M1B = 63         # [48, 1]
FB_M2W = 64         # [48, 16]
FB_M2B = 80         # [16, 1]
FB_M3W = 81         # [16, 1]
FB_M3B = 82         # [1, 1]
FB_LINB = 83        # [33, 1]
FBW = 84
# Wb blob column map (bf16 [128, WBW])
WB_WLS = 0          # [128, 5*128]
WB_WRS = 640        # [128, 5*128]
WB_GATE = 1280      # [128, 6]
WB_WREL = 1286      # [128, 3]
WB_WROOT = 1289     # [128, 3]
WBW = 1292


# ----------------------------------------------------------------- host prep
def wrap16(idx):
    n = idx.shape[0]
    return np.ascontiguousarray(idx.reshape(n // 16, 16).T.astype(np.int16))


def window_major_edges(src, dst, nwin, cmax, sentinel):
    order = np.argsort(dst, kind='stable')
    src, dst = src[order], dst[order]
    nslot = nwin * cmax * 128
    sp = np.full(nslot, sentinel, np.int64)
    dl = np.full((128, nwin * cmax), -1.0, np.float32)
    for w in range(nwin):
        lo = np.searchsorted(dst, w * 128)
        hi = np.searchsorted(dst, (w + 1) * 128)
        if hi <= lo:
            continue
        assert hi - lo <= cmax * 128, f"window {w}: {hi-lo}"
        base = w * cmax * 128
        sp[base:base + (hi - lo)] = src[lo:hi]
        for k in range(cmax):
            a = lo + k * 128
            if a >= hi:
                break
            b = min(hi, a + 128)
            dl[:b - a, w * cmax + k] = (dst[a:b] - w * 128).astype(np.float32)
    return sp, dl


def host_prep(inputs):
    h = np.asarray(inputs['h'], np.float32)
    ei = np.asarray(inputs['edge_index'], np.int64)
    sei = np.asarray(inputs['sub_edge_index'], np.int64)

    src1 = np.concatenate([ei[0], np.arange(N)])
    dst1 = np.concatenate([ei[1], np.arange(N)])
    cnt_w = np.bincount(dst1 // 128, minlength=64)
    CMAX1 = int(np.ceil(cnt_w.max() / 128))

    s1 = []
    for c in range(NC):
        m = (dst1 >= c * S1SLICE) & (dst1 < (c + 1) * S1SLICE)
        sp, dl = window_major_edges(src1[m], dst1[m] - c * S1SLICE,
                                    S1W, CMAX1, NP - 1)
        s1.append((wrap16(sp), dl.astype(BF)))

    hT = np.zeros((16, NP), BF)
    hT[:, :N] = h.T.astype(BF)

    ssrc = (sei[0].reshape(P, ES) - (np.arange(P) * N)[:, None])
    sdst = (sei[1].reshape(P, ES) - (np.arange(P) * N)[:, None])
    CMAX2 = 1
    for g in range(P):
        CMAX2 = max(CMAX2, int(np.ceil(
            np.bincount(sdst[g] // 128, minlength=W2).max() / 128)))

    s2src, s2dl = [], []
    for c in range(NC):
        a_s, a_d = [], []
        for j in range(GPC):
            g = c * GPC + j
            sp, dl = window_major_edges(ssrc[g], sdst[g], W2, CMAX2, NP - 1)
            a_s.append(wrap16(sp))
            a_d.append(dl)
        s2src.append(np.ascontiguousarray(np.concatenate(a_s, axis=1)))
        s2dl.append(np.ascontiguousarray(
            np.concatenate(a_d, axis=1).astype(BF)))

    W = {k: np.asarray(inputs[k], np.float32) for k in inputs}

    Fb = np.zeros((128, FBW), np.float32)
    Fb[:, FB_BLA] = W['bl_a']
    Fb[:, FB_BLS:FB_BLS + 5] = W['bl_s'].T
    Fb[:, FB_BREL:FB_BREL + 3] = np.tile(W['pool_brel'][None, :], (128, 1))
    Fb[:, FB_NW] = W['norm_w']
    Fb[:, FB_NB] = W['norm_b']
    Fb[:, FB_NMS] = W['norm_ms']
    Fb[:, FB_LINW:FB_LINW + 3] = W['lin_w'].reshape(3, 128).T
    Fb[0:33, FB_M1W:FB_M1W + 48] = W['m1_w']
    Fb[0:48, FB_M1B] = W['m1_b']
    Fb[0:48, FB_M2W:FB_M2W + 16] = W['m2_w']
    Fb[0:16, FB_M2B] = W['m2_b']
    Fb[0:16, FB_M3W] = W['m3_w'][:, 0]
    Fb[0, FB_M3B] = W['m3_b'][0]
    Fb[0:33, FB_LINB] = W['lin_b'][0]

    Wb = np.zeros((128, WBW), BF)
    Wb[:, WB_WLS:WB_WLS + 640] = np.ascontiguousarray(
        W['Wl_s'].transpose(1, 0, 2)).reshape(128, 640).astype(BF)
    Wb[:, WB_WRS:WB_WRS + 640] = np.ascontiguousarray(
        W['Wr_s'].transpose(1, 0, 2)).reshape(128, 640).astype(BF)
    Wb[:, WB_GATE:WB_GATE + 6] = W['gate_w'].T.astype(BF)
    Wb[:, WB_WREL:WB_WREL + 3] = W['pool_wrel'].T.astype(BF)
    Wb[:, WB_WROOT:WB_WROOT + 3] = W['pool_wroot'].T.astype(BF)

    WA = np.zeros((16, 256), BF)
    WA[:, 0:128] = W['Wl_a'].astype(BF)
    WA[:, 128:256] = W['Wr_a'].astype(BF)

    base = dict(hT=hT, Wb=Wb, WA=WA, Fb=Fb)
    per_core = []
    for c in range(NC):
        m = dict(base)
        m['s1_src'], m['s1_dl'] = s1[c]
        m['s2_src'], m['s2_dl'] = s2src[c], s2dl[c]
        per_core.append(m)
    return per_core, (CMAX1, CMAX2)


# ---------------------------------------------------------------- the kernel
def build_nc(CMAX1, CMAX2, debug=False):
    NP1 = S1W * CMAX1 * 128
    NP2 = W2 * CMAX2 * 128
    PR1 = S1W * CMAX1
    PR2 = W2 * CMAX2

    nc = bacc.Bacc("TRN2", target_bir_lowering=False, debug=False,
                   num_devices=NC)
    f32, bf16 = dt.float32, dt.bfloat16

    def inp(name, shape, d=f32):
        return nc.dram_tensor(name, shape, d, kind="ExternalInput")

    t_hT = inp('hT', [16, NP], bf16)
    t_s1src = inp('s1_src', [16, NP1 // 16], dt.int16)
    t_s1dl = inp('s1_dl', [128, PR1], bf16)
    t_s2src = inp('s2_src', [16, GPC * NP2 // 16], dt.int16)
    t_s2dl = inp('s2_dl', [128, GPC * PR2], bf16)
    t_Wb = inp('Wb', [128, WBW], bf16)
    t_WA = inp('WA', [16, 256], bf16)
    t_Fb = inp('Fb', [128, FBW])

    out = nc.dram_tensor('out', [1, 1], f32, kind="ExternalOutput")
    dbg = {}
    if debug:
        for nm, shp in (('d_x1T', [128, NP]), ('d_ro', [33, 384]),
                        ('d_sc0', [128, W2]), ('d_na0', [128, W2])):
            dbg[nm] = nc.dram_tensor(nm, shp, f32, kind="ExternalOutput")

    with tile.TileContext(nc, trace_sim=False) as tc:
        cp = tc.alloc_tile_pool(name="const", bufs=1)
        big = tc.alloc_tile_pool(name="big", bufs=1)
        wp = tc.alloc_tile_pool(name="wk", bufs=2)
        pp = tc.alloc_tile_pool(name="ps", bufs=2, space="PSUM")
        dp = tc.alloc_tile_pool(name="dram", bufs=1, space="DRAM")

        # ------------- packed constants
        Wb = cp.tile([128, WBW], bf16, tag="Wb")
        nc.sync.dma_start(Wb[:], t_Wb.ap())
        WA = cp.tile([16, 256], bf16, tag="WA")
        nc.sync.dma_start(WA[:], t_WA.ap())
        Fb = cp.tile([128, FBW], f32, tag="Fb")
        nc.sync.dma_start(Fb[:], t_Fb.ap())

        def WlS(i):
            return Wb[:, WB_WLS + i * 128:WB_WLS + (i + 1) * 128]

        def WrS(i):
            return Wb[:, WB_WRS + i * 128:WB_WRS + (i + 1) * 128]

        gate = Wb[:, WB_GATE:WB_GATE + 6]
        wrel = Wb[:, WB_WREL:WB_WREL + 3]
        wroot = Wb[:, WB_WROOT:WB_WROOT + 3]
        WlA = WA[:, 0:128]
        WrA = WA[:, 128:256]
        blA = Fb[:, FB_BLA:FB_BLA + 1]
        blS = Fb[:, FB_BLS:FB_BLS + 5]
        brel = Fb[:, FB_BREL:FB_BREL + 3]
        nw = Fb[:, FB_NW:FB_NW + 1]
        nb = Fb[:, FB_NB:FB_NB + 1]
        nms = Fb[:, FB_NMS:FB_NMS + 1]
        linw = Fb[:, FB_LINW:FB_LINW + 3]
        m1w = Fb[0:33, FB_M1W:FB_M1W + 48]
        m1b = Fb[0:48, FB_M1B:FB_M1B + 1]
        m2w = Fb[0:48, FB_M2W:FB_M2W + 16]
        m2b = Fb[0:16, FB_M2B:FB_M2B + 1]
        m3w = Fb[0:16, FB_M3W:FB_M3W + 1]
        m3b = Fb[0:1, FB_M3B:FB_M3B + 1]
        linb = Fb[0:33, FB_LINB:FB_LINB + 1]

        # ------------- generated constants
        iota = cp.tile([128, 128], f32)
        nc.gpsimd.iota(iota[:], pattern=[[1, 128]], base=0,
                       channel_multiplier=0,
                       allow_small_or_imprecise_dtypes=True)
        ones128 = cp.tile([128, 128], f32)
        nc.vector.memset(ones128[:], 1.0)
        identf = cp.tile([128, 128], f32)
        nc.gpsimd.affine_select(identf[:], ones128[:], pattern=[[-1, 128]],
                                compare_op=OP.is_equal, fill=0.0, base=0,
                                channel_multiplier=1)
        ident = cp.tile([128, 128], bf16)
        nc.vector.tensor_copy(out=ident[:], in_=identf[:])
        nid = cp.tile([128, W2], f32)
        nc.gpsimd.iota(nid[:], pattern=[[128, W2]], base=0,
                       channel_multiplier=1,
                       allow_small_or_imprecise_dtypes=True)
        na0c = cp.tile([128, W2], f32)
        nc.vector.tensor_scalar(out=na0c[:], in0=nid[:],
                                scalar1=float(N) - 0.5, scalar2=None,
                                op0=OP.is_ge)
        nc.vector.tensor_scalar(out=na0c[:], in0=na0c[:], scalar1=-1.0,
                                scalar2=1.0, op0=OP.mult, op1=OP.add)
        probei = cp.tile([1, NPROBE], f32)
        nc.gpsimd.iota(probei[:], pattern=[[1, NPROBE]], base=1,
                       channel_multiplier=0,
                       allow_small_or_imprecise_dtypes=True)
        ones_c = cp.tile([128, 1], f32); nc.vector.memset(ones_c[:], 1.0)
        ones_cb = cp.tile([128, 1], bf16); nc.vector.memset(ones_cb[:], 1.0)
        ones_r = cp.tile([1, 128], f32); nc.vector.memset(ones_r[:], 1.0)
        ones_rb = cp.tile([1, 128], bf16); nc.vector.memset(ones_rb[:], 1.0)

        xT = big.tile([128, NP], bf16, tag="xT")       # current features^T
        rows = big.tile([128, W2, 128], bf16, tag="rows")
        ro2s = cp.tile([128, 3, GPC], f32)
        ro1s = cp.tile([128, 3], f32)
        xrows1 = dp.tile([NP, 128], bf16)              # stage-1 row table
        xrows2 = dp.tile([NP, 128], bf16)              # stage-2 scratch table
        utab = dp.tile([NP, 128], bf16)                # u table (DRAM)

        # =========================================================== helpers
        def transpose_pass(srcT, dst):
            """srcT [128, NP] bf16 -> rows [128, W2, 128]; optionally DMA to
            a DRAM row table."""
            for w in range(W2):
                tp = pp.tile([128, 128], bf16, space="PSUM", tag="psA")
                nc.tensor.transpose(tp[:], srcT[:, w * 128:(w + 1) * 128],
                                    ident[:])
                nc.scalar.copy(out=rows[:, w, :], in_=tp[:])
            if dst is not None:
                nc.sync.dma_start(
                    dst[:].rearrange("(w p) e -> p w e", p=128), rows[:])

        rowbounce = dp.tile([1, NP], bf16)
        frow = cp.tile([1, NP], bf16)

        def row_from_grid(gtile, rowt, wn):
            nc.gpsimd.dma_start(
                rowbounce[0:1, 0:wn * 128].rearrange("o (w p) -> o p w",
                                                     p=128), gtile[:])
            nc.sync.dma_start(rowt[0:1, 0:wn * 128],
                              rowbounce[0:1, 0:wn * 128])

        def pe_bcast_col(val11):
            """[1,1] f32 -> psum [128,1]."""
            ps = pp.tile([128, 1], f32, space="PSUM", tag="psB")
            nc.tensor.matmul(ps[:], lhsT=ones_r[:], rhs=val11[:],
                             start=True, stop=True)
            return ps

        def readout(xmT, gate_i, na_g, f_g, slot, use_mask):
            """global-attention readout -> slot [128,1].
            a = softmax over alive of (f * (gate.x)); r = sum a*xm via rows."""
            gx = pp.tile([128, W2], f32, space="PSUM", tag="psB")
            for w in range(W2):
                nc.tensor.matmul(gx[:, w:w + 1],
                                 lhsT=xmT[:, w * 128:(w + 1) * 128],
                                 rhs=gate[:, gate_i:gate_i + 1],
                                 start=True, stop=True)
            gxs = wp.tile([128, W2], f32, tag="gxs")
            if use_mask:
                nc.vector.tensor_tensor(out=gxs[:], in0=gx[:], in1=f_g[:],
                                        op=OP.mult)
            else:
                nc.vector.tensor_copy(out=gxs[:], in_=gx[:])
            ex = wp.tile([128, W2], f32, tag="ex")
            nc.scalar.activation(ex[:], gxs[:], AF.Exp)
            nc.vector.tensor_tensor(out=ex[:], in0=ex[:],
                                    in1=(na_g[:] if use_mask else na0c[:]),
                                    op=OP.mult)
            sm = wp.tile([128, 1], f32, tag="sm")
            nc.vector.tensor_reduce(out=sm[:], in_=ex[:], axis=AX.X, op=OP.add)
            tot = pp.tile([1, 1], f32, space="PSUM", tag="psB")
            nc.tensor.matmul(tot[:], lhsT=sm[:], rhs=ones_c[:],
                             start=True, stop=True)
            rden = wp.tile([1, 1], f32, tag="rden")
            nc.vector.reciprocal(rden[:], tot[:])
            rdb = pe_bcast_col(rden)
            ab = wp.tile([128, W2], bf16, tag="ab")
            nc.vector.tensor_scalar(out=ab[:], in0=ex[:], scalar1=rdb[:, 0:1],
                                    scalar2=None, op0=OP.mult)
            rps = pp.tile([128, 1], f32, space="PSUM", tag="psB")
            for w in range(W2):
                nc.tensor.matmul(rps[:], lhsT=rows[:, w, :],
                                 rhs=ab[:, w:w + 1],
                                 start=(w == 0), stop=(w == W2 - 1))
            nc.vector.tensor_copy(out=slot, in_=rps[:])

        # ====================================================== stage 1
        s1pool = tc.alloc_tile_pool(name="s1", bufs=1)
        s1src = s1pool.tile([128, NP1 // 16], dt.int16, tag="s1src")
        for k in range(8):
            nc.sync.dma_start(s1src[16 * k:16 * (k + 1), :], t_s1src.ap())
        s1dlb = s1pool.tile([128, PR1], bf16, tag="s1dlb")
        nc.sync.dma_start(s1dlb[:], t_s1dl.ap())
        s1dl = s1pool.tile([128, PR1], f32, tag="s1dl")
        nc.vector.tensor_copy(out=s1dl[:], in_=s1dlb[:])
        eq1 = s1pool.tile([128, PR1, 128], bf16)
        for pr in range(PR1):
            nc.vector.tensor_scalar(out=eq1[:, pr, :], in0=iota[:],
                                    scalar1=s1dl[:, pr:pr + 1], scalar2=None,
                                    op0=OP.is_equal)

        # reciprocal in-degree of this core's dst slice (same for all layers)
        degp1 = pp.tile([128, S1W], f32, space="PSUM", tag="psB")
        for w in range(S1W):
            for k in range(CMAX1):
                pr = w * CMAX1 + k
                nc.tensor.matmul(degp1[:, w:w + 1], lhsT=eq1[:, pr, :],
                                 rhs=ones_cb[:], start=(k == 0),
                                 stop=(k == CMAX1 - 1))
        rdeg1g = wp.tile([128, S1W], f32, tag="rdeg1g")
        nc.vector.tensor_scalar(out=rdeg1g[:], in0=degp1[:], scalar1=1.0,
                                scalar2=None, op0=OP.max)
        nc.vector.reciprocal(rdeg1g[:], rdeg1g[:])
        row_from_grid(rdeg1g, frow, S1W)
        rdegb = s1pool.tile([128, S1SLICE], bf16, tag="rdegb")
        for t in range(S1SLICE // 512):
            sl = slice(t * 512, (t + 1) * 512)
            ob = pp.tile([128, 512], f32, space="PSUM", tag="psA")
            nc.tensor.matmul(ob[:], lhsT=ones_rb[:], rhs=frow[0:1, sl],
                             start=True, stop=True)
            nc.scalar.copy(out=rdegb[:, sl], in_=ob[:])

        # publish h rows (feature-padded with zeros) for the layer-0 gather
        nc.vector.memset(xT[:], 0.0)
        nc.sync.dma_start(xT[0:16, :], t_hT.ap())
        transpose_pass(xT, xrows1)

        agb_in = dp.tile([128, S1SLICE], bf16)
        agb_out = dp.tile([128 * NC, S1SLICE], bf16)
        meanT = big.tile([128, NP], bf16, tag="meanT")

        def s1_layer(layer):
            srcT = xT
            aggp = pp.tile([128, S1W, 128], f32, space="PSUM", tag="pagg")
            for half in range(2):
                msgs = s1pool.tile([128, PR1 // 2, 128], bf16, tag="msgs1")
                nc.gpsimd.dma_gather(
                    out_ap=msgs[:], in_ap=xrows1[:],
                    idxs_ap=s1src[:, half * (NP1 // 32):(half + 1) * (NP1 // 32)],
                    num_idxs=NP1 // 2, num_idxs_reg=NP1 // 2, elem_size=128,
                    single_packet=False)
                for w in range(S1W // 2 * half, S1W // 2 * (half + 1)):
                    for k in range(CMAX1):
                        pr = w * CMAX1 + k
                        lpr = pr - half * (PR1 // 2)
                        nc.tensor.matmul(aggp[:, w, :], lhsT=msgs[:, lpr, :],
                                         rhs=eq1[:, pr, :], start=(k == 0),
                                         stop=(k == CMAX1 - 1))
            mslice = wp.tile([128, S1SLICE], bf16, tag="mslice")
            nc.vector.tensor_tensor(
                out=mslice[:], in0=aggp[:].rearrange("p w e -> p (w e)"),
                in1=rdegb[:], op=OP.mult)
            nc.sync.dma_start(agb_in[:], mslice[:])
            nc.gpsimd.collective_compute(
                "AllGather", OP.bypass, replica_groups=[list(range(NC))],
                ins=[agb_in.opt()], outs=[agb_out.opt()])
            # reassemble meanT full [128, NP]
            nc.sync.dma_start(
                meanT[:].rearrange("p (c s) -> p c s", c=NC),
                agb_out[:].rearrange("(c p) s -> p c s", p=128))
            Wl = WlA if layer == 0 else WlS(layer - 1)
            Wr = WrA if layer == 0 else WrS(layer - 1)
            bl = blA if layer == 0 else blS[:, layer - 1:layer]
            kdim = 16 if layer == 0 else 128
            for t in range(NP // 512):
                sl = slice(t * 512, (t + 1) * 512)
                xp = pp.tile([128, 512], f32, space="PSUM", tag="psA")
                nc.tensor.matmul(xp[:], lhsT=Wl[0:kdim, :],
                                 rhs=meanT[0:kdim, sl], start=True, stop=False)
                nc.tensor.matmul(xp[:], lhsT=Wr[0:kdim, :],
                                 rhs=srcT[0:kdim, sl], start=False, stop=True)
                nc.scalar.activation(xT[:, sl], xp[:], AF.Tanh, bias=bl)

        def gnorm():
            mu = wp.tile([128, 1], f32, tag="mu")
            nc.vector.tensor_reduce(out=mu[:], in_=xT[:, 0:N], axis=AX.X,
                                    op=OP.add)
            nc.vector.tensor_scalar(out=mu[:], in0=mu[:], scalar1=1.0 / N,
                                    scalar2=None, op0=OP.mult)
            mums = wp.tile([128, 1], f32, tag="mums")
            nc.vector.tensor_tensor(out=mums[:], in0=mu[:], in1=nms[:],
                                    op=OP.mult)
            o = s1pool.tile([128, NP], bf16, tag="onorm")
            nc.vector.tensor_scalar(out=o[:], in0=xT[:], scalar1=mums[:],
                                    scalar2=None, op0=OP.subtract)
            var = wp.tile([128, 1], f32, tag="var")
            sq = s1pool.tile([128, NP], bf16, tag="sqnorm")
            nc.vector.tensor_tensor(out=sq[:], in0=o[:], in1=o[:], op=OP.mult)
            nc.vector.tensor_reduce(out=var[:], in_=sq[:, 0:N], axis=AX.X,
                                    op=OP.add)
            nc.vector.tensor_scalar(out=var[:], in0=var[:], scalar1=1.0 / N,
                                    scalar2=1e-5, op0=OP.mult, op1=OP.add)
            rstd = wp.tile([128, 1], f32, tag="rstd")
            nc.vector.reciprocal(rstd[:], var[:])
            nc.scalar.activation(rstd[:], rstd[:], AF.Sqrt)
            sc = wp.tile([128, 1], f32, tag="scn")
            nc.vector.tensor_tensor(out=sc[:], in0=rstd[:], in1=nw[:],
                                    op=OP.mult)
            nc.vector.tensor_scalar(out=xT[:], in0=o[:], scalar1=sc[:],
                                    scalar2=nb[:], op0=OP.mult, op1=OP.add)

        for layer in range(3):
            s1_layer(layer)
            transpose_pass(xT, None)
            readout(xT, layer, None, None, ro1s[:, layer:layer + 1],
                    use_mask=False)
            if layer < 2:
                gnorm()
            transpose_pass(xT, xrows1)

        if dbg:
            nc.gpsimd.dma_start(dbg['d_x1T'].ap(), xT[:])

        s1pool.release()

        # ====================================================== stage 2
        s2pool = tc.alloc_tile_pool(name="s2", bufs=1)
        s2g = tc.alloc_tile_pool(name="s2g", bufs=2)
        s2src = s2pool.tile([128, GPC * NP2 // 16], dt.int16, tag="s2src")
        for k in range(8):
            nc.sync.dma_start(s2src[16 * k:16 * (k + 1), :], t_s2src.ap())
        s2dlb = s2pool.tile([128, GPC * PR2], bf16, tag="s2dlb")
        nc.sync.dma_start(s2dlb[:], t_s2dl.ap())
        s2dl = s2pool.tile([128, GPC * PR2], f32, tag="s2dl")
        nc.vector.tensor_copy(out=s2dl[:], in_=s2dlb[:])

        xgT = s2pool.tile([128, NP], bf16, tag="xgT")
        eq2 = s2pool.tile([128, PR2, 128], bf16, tag="eq2")
        rbc = s2pool.tile([128, NP], bf16, tag="rbc")
        ones_pr = s2pool.tile([128, PR2], bf16, tag="ones_pr")
        nc.vector.memset(ones_pr[:], 1.0)

        for j in range(GPC):
            # per-graph eq tiles (unscaled one-hot)
            for pr in range(PR2):
                nc.vector.tensor_scalar(
                    out=eq2[:, pr, :], in0=iota[:],
                    scalar1=s2dl[:, j * PR2 + pr:j * PR2 + pr + 1],
                    scalar2=None, op0=OP.is_equal)
            nc.vector.tensor_copy(out=xgT[:], in_=xT[:])
            na_g = cp.tile([128, W2], f32, tag=f"nag{j}")
            f_g = cp.tile([128, W2], f32, tag=f"fg{j}")
            nc.vector.tensor_copy(out=na_g[:], in_=na0c[:])

            for l in range(3):
                li = 2 + l
                # ---- messages gather (stage-1 table at l=0, else own)
                msgs = s2g.tile([128, PR2, 128], bf16, tag="m2")
                nc.gpsimd.dma_gather(
                    out_ap=msgs[:], in_ap=(xrows1[:] if l == 0 else xrows2[:]),
                    idxs_ap=s2src[:, j * (NP2 // 16):(j + 1) * (NP2 // 16)],
                    num_idxs=NP2, num_idxs_reg=NP2, elem_size=128,
                    single_packet=False)
                # ---- reciprocal alive-in-degree -> rbc [128, NP]
                if l == 0:
                    ind = ones_pr          # all edges live before 1st pool
                else:
                    ind = wp.tile([128, PR2], bf16, tag="ind")
                    nc.vector.tensor_scalar(out=ind[:],
                                            in0=msgs[:, :, 0:1].rearrange(
                                                "p c o -> p (c o)"),
                                            scalar1=0.0, scalar2=None,
                                            op0=OP.not_equal)
                degp = pp.tile([128, W2], f32, space="PSUM", tag="psB")
                for w in range(W2):
                    for k in range(CMAX2):
                        pr = w * CMAX2 + k
                        nc.tensor.matmul(
                            degp[:, w:w + 1], lhsT=eq2[:, pr, :],
                            rhs=ind[:, pr:pr + 1], start=(k == 0),
                            stop=(k == CMAX2 - 1))
                rdeg = wp.tile([128, W2], f32, tag="rdeg")
                nc.vector.tensor_scalar(out=rdeg[:], in0=degp[:],
                                        scalar1=1.0, scalar2=None,
                                        op0=OP.max)
                nc.vector.reciprocal(rdeg[:], rdeg[:])
                nc.vector.tensor_tensor(out=rdeg[:], in0=rdeg[:],
                                        in1=na_g[:], op=OP.mult)
                row_from_grid(rdeg, frow, W2)
                for t in range(NP // 512):
                    sl = slice(t * 512, (t + 1) * 512)
                    ob = pp.tile([128, 512], f32, space="PSUM", tag="psA")
                    nc.tensor.matmul(ob[:], lhsT=ones_rb[:],
                                     rhs=frow[0:1, sl], start=True,
                                     stop=True)
                    nc.scalar.copy(out=rbc[:, sl], in_=ob[:])
                # ---- aggregation matmuls + fused mean scale
                for grp in range(8):
                    agp = pp.tile([128, 8, 128], f32, space="PSUM",
                                  tag="pagg")
                    for wi in range(8):
                        w = grp * 8 + wi
                        for k in range(CMAX2):
                            pr = w * CMAX2 + k
                            nc.tensor.matmul(agp[:, wi, :],
                                             lhsT=msgs[:, pr, :],
                                             rhs=eq2[:, pr, :],
                                             start=(k == 0),
                                             stop=(k == CMAX2 - 1))
                    sl = slice(grp * 1024, (grp + 1) * 1024)
                    nc.vector.tensor_tensor(
                        out=meanT[:, sl],
                        in0=agp[:].rearrange("p w e -> p (w e)"),
                        in1=rbc[:, sl], op=OP.mult)
                # ---- x' = tanh(Wl.T meanT + Wr.T xm + bl)
                for t in range(NP // 512):
                    sl = slice(t * 512, (t + 1) * 512)
                    xp = pp.tile([128, 512], f32, space="PSUM", tag="psA")
                    nc.tensor.matmul(xp[:], lhsT=WlS(li), rhs=meanT[:, sl],
                                     start=True, stop=False)
                    nc.tensor.matmul(xp[:], lhsT=WrS(li), rhs=xgT[:, sl],
                                     start=False, stop=True)
                    nc.scalar.activation(xgT[:, sl], xp[:], AF.Tanh,
                                         bias=blS[:, li:li + 1])
                # ---- u/uroot/gx grids
                ug = pp.tile([128, W2, 3], f32, space="PSUM", tag="psB")
                wcols = cp.tile([128, 3], bf16, tag=f"wcols{j}_{l}")
                nc.vector.tensor_copy(out=wcols[:, 0:1], in_=wrel[:, l:l + 1])
                nc.vector.tensor_copy(out=wcols[:, 1:2],
                                      in_=wroot[:, l:l + 1])
                nc.vector.tensor_copy(out=wcols[:, 2:3],
                                      in_=gate[:, 3 + l:4 + l])
                for w in range(W2):
                    nc.tensor.matmul(ug[:, w, :],
                                     lhsT=xgT[:, w * 128:(w + 1) * 128],
                                     rhs=wcols[:], start=True, stop=True)
                ugs = wp.tile([128, W2, 3], f32, tag="ugs")
                nc.vector.tensor_copy(out=ugs[:], in_=ug[:])
                # write u table (strided, cast bf16)
                nc.gpsimd.dma_start(
                    utab[:].rearrange("(w p) e -> p w e", p=128)[:, :, 0:1],
                    ugs[:, :, 0:1])
                # ---- score pass
                umsg = s2g.tile([128, PR2, 128], bf16, tag="m2")
                nc.gpsimd.dma_gather(
                    out_ap=umsg[:], in_ap=utab[:],
                    idxs_ap=s2src[:, j * (NP2 // 16):(j + 1) * (NP2 // 16)],
                    num_idxs=NP2, num_idxs_reg=NP2, elem_size=128,
                    single_packet=False)
                scp = pp.tile([128, W2], f32, space="PSUM", tag="psB")
                for w in range(W2):
                    for k in range(CMAX2):
                        pr = w * CMAX2 + k
                        nc.tensor.matmul(scp[:, w:w + 1], lhsT=eq2[:, pr, :],
                                         rhs=umsg[:, pr, 0:1],
                                         start=(k == 0),
                                         stop=(k == CMAX2 - 1))
                score = cp.tile([128, W2], f32, tag=f"score{j}_{l}")
                nc.vector.tensor_tensor(out=score[:], in0=scp[:],
                                        in1=ugs[:, :, 1], op=OP.add)
                nc.vector.tensor_scalar(out=score[:], in0=score[:],
                                        scalar1=brel[:, l:l + 1],
                                        scalar2=None, op0=OP.add)
                if dbg and j == 0 and l == 0:
                    nc.gpsimd.dma_start(dbg['d_sc0'].ap(), score[:])
                # ---- top-k threshold via multiprobe
                sm_ = cp.tile([128, W2], f32, tag=f"smask{j}_{l}")
                nc.vector.tensor_tensor(out=sm_[:], in0=score[:], in1=na_g[:],
                                        op=OP.mult)
                pen = wp.tile([128, W2], f32, tag="pen")
                nc.vector.tensor_scalar(out=pen[:], in0=na_g[:],
                                        scalar1=1e6, scalar2=-1e6,
                                        op0=OP.mult, op1=OP.add)
                nc.vector.tensor_tensor(out=sm_[:], in0=sm_[:], in1=pen[:],
                                        op=OP.add)
                lo = cp.tile([1, 1], f32, tag=f"lo{j}_{l}")
                st = cp.tile([1, 1], f32, tag=f"st{j}_{l}")
                nc.vector.memset(lo[:], -16.0)
                nc.vector.memset(st[:], 32.0 / NPROBE)
                kk = float(KS[l])  # per-graph keep count
                for it in range(NITER):
                    pr_ = wp.tile([1, NPROBE], f32, tag="pr_")
                    nc.vector.tensor_scalar(out=pr_[:], in0=probei[:],
                                            scalar1=st[:, 0:1], scalar2=None,
                                            op0=OP.mult)
                    nc.vector.tensor_scalar(out=pr_[:], in0=pr_[:],
                                            scalar1=lo[:, 0:1], scalar2=None,
                                            op0=OP.add)
                    pb = pp.tile([128, NPROBE], f32, space="PSUM", tag="psB")
                    nc.tensor.matmul(pb[:], lhsT=ones_r[:], rhs=pr_[:],
                                     start=True, stop=True)
                    cmp_ = wp.tile([128, NPROBE, W2], f32, tag="cmp_")
                    nc.vector.tensor_tensor(
                        out=cmp_[:],
                        in0=sm_[:].rearrange("p (o w) -> p o w", o=1)
                            .to_broadcast([128, NPROBE, W2]),
                        in1=pb[:].rearrange("p (r o) -> p r o", o=1)
                            .to_broadcast([128, NPROBE, W2]),
                        op=OP.is_ge)
                    cnt = wp.tile([128, NPROBE], f32, tag="cnt")
                    nc.vector.tensor_reduce(out=cnt[:], in_=cmp_[:],
                                            axis=AX.X, op=OP.add)
                    cs = pp.tile([1, NPROBE], f32, space="PSUM", tag="psB")
                    nc.tensor.matmul(cs[:], lhsT=ones_c[:], rhs=cnt[:],
                                     start=True, stop=True)
                    sges = wp.tile([1, NPROBE], f32, tag="sges")
                    nc.vector.tensor_scalar(out=sges[:], in0=cs[:],
                                            scalar1=kk - 0.5, scalar2=None,
                                            op0=OP.is_ge)
                    s8 = wp.tile([1, 1], f32, tag="s8")
                    nc.vector.tensor_reduce(out=s8[:], in_=sges[:],
                                            axis=AX.X, op=OP.add)
                    nc.vector.tensor_tensor(out=s8[:], in0=s8[:],
                                            in1=st[:], op=OP.mult)
                    nc.vector.tensor_tensor(out=lo[:], in0=lo[:], in1=s8[:],
                                            op=OP.add)
                    nc.vector.tensor_scalar(out=st[:], in0=st[:],
                                            scalar1=1.0 / NPROBE,
                                            scalar2=None, op0=OP.mult)
                thr = wp.tile([1, 1], f32, tag="thr")
                nc.vector.tensor_scalar(out=thr[:], in0=st[:],
                                        scalar1=float(NPROBE) / 2,
                                        scalar2=None, op0=OP.mult)
                nc.vector.tensor_tensor(out=thr[:], in0=thr[:], in1=lo[:],
                                        op=OP.add)
                thb = pe_bcast_col(thr)
                nc.vector.tensor_scalar(out=na_g[:], in0=sm_[:],
                                        scalar1=thb[:, 0:1], scalar2=None,
                                        op0=OP.is_ge)
                if dbg and j == 0 and l == 0:
                    nc.gpsimd.dma_start(dbg['d_na0'].ap(), na_g[:])
                # f = na * tanh(score)
                nc.scalar.activation(f_g[:], score[:], AF.Tanh)
                nc.vector.tensor_tensor(out=f_g[:], in0=f_g[:], in1=na_g[:],
                                        op=OP.mult)
                # ---- mask xgT in place: xm = x * f  (column scale).
                # At l==2 nothing downstream needs xmT densely (readout
                # folds f into the attention weights), so skip the scale.
                if l < 2:
                    row_from_grid(f_g, frow, W2)
                    for t in range(NP // 512):
                        sl = slice(t * 512, (t + 1) * 512)
                        ob = pp.tile([128, 512], f32, space="PSUM", tag="psA")
                        nc.tensor.matmul(ob[:], lhsT=ones_rb[:],
                                         rhs=frow[0:1, sl], start=True,
                                         stop=True)
                        nc.vector.tensor_tensor(out=xgT[:, sl],
                                                in0=xgT[:, sl],
                                                in1=ob[:], op=OP.mult)
                # ---- rows of xm (+ DMA for next layer's gather)
                transpose_pass(xgT, xrows2 if l < 2 else None)
                # ---- readout (gx grid = ug[:,:,2] is pre-mask -> fold f)
                gxg = wp.tile([128, W2], f32, tag="gxg")
                nc.vector.tensor_tensor(out=gxg[:], in0=ugs[:, :, 2],
                                        in1=f_g[:], op=OP.mult)
                ex = wp.tile([128, W2], f32, tag="ex")
                nc.scalar.activation(ex[:], gxg[:], AF.Exp)
                nc.vector.tensor_tensor(out=ex[:], in0=ex[:], in1=na_g[:],
                                        op=OP.mult)
                smr = wp.tile([128, 1], f32, tag="smr")
                nc.vector.tensor_reduce(out=smr[:], in_=ex[:], axis=AX.X,
                                        op=OP.add)
                tot = pp.tile([1, 1], f32, space="PSUM", tag="psB")
                nc.tensor.matmul(tot[:], lhsT=smr[:], rhs=ones_c[:],
                                 start=True, stop=True)
                rden = wp.tile([1, 1], f32, tag="rden")
                nc.vector.reciprocal(rden[:], tot[:])
                rdb = pe_bcast_col(rden)
                ab = wp.tile([128, W2], bf16, tag="ab")
                nc.vector.tensor_scalar(out=ab[:], in0=ex[:],
                                        scalar1=rdb[:, 0:1], scalar2=None,
                                        op0=OP.mult)
                if l == 2:
                    nc.vector.tensor_tensor(out=ab[:], in0=ab[:],
                                            in1=f_g[:], op=OP.mult)
                rps = pp.tile([128, 1], f32, space="PSUM", tag="psB")
                for w in range(W2):
                    nc.tensor.matmul(rps[:], lhsT=rows[:, w, :],
                                     rhs=ab[:, w:w + 1], start=(w == 0),
                                     stop=(w == W2 - 1))
                nc.vector.tensor_copy(out=ro2s[:, l, j:j + 1], in_=rps[:])

        # ====================================================== final MLP
        rob_in = dp.tile([GPC, 3 * 128], f32)
        rob_out = dp.tile([P, 3 * 128], f32)
        for j in range(GPC):
            nc.sync.dma_start(
                rob_in[j:j + 1, :].rearrange("g (l p) -> p (g l)", p=128),
                ro2s[:, :, j])
        nc.gpsimd.collective_compute(
            "AllGather", OP.bypass, replica_groups=[list(range(NC))],
            ins=[rob_in.opt()], outs=[rob_out.opt()])
        roall = cp.tile([32, 384], f32)
        nc.sync.dma_start(roall[:], rob_out[:])
        roT = cp.tile([128, 3, 33], f32)
        nc.vector.tensor_copy(out=roT[:, :, 0:1],
                              in_=ro1s[:].rearrange("p (l o) -> p l o", o=1))
        for lblk in range(3):
            tp = pp.tile([128, 32], f32, space="PSUM", tag="psB")
            nc.tensor.transpose(tp[:, 0:32],
                                roall[:, lblk * 128:(lblk + 1) * 128],
                                identf[0:32, 0:32])
            nc.vector.tensor_copy(out=roT[:, lblk, 1:33], in_=tp[:, 0:32])
        zp = pp.tile([33, 1], f32, space="PSUM", tag="psB")
        for lblk in range(3):
            nc.tensor.matmul(zp[:], lhsT=roT[:, lblk, :],
                             rhs=linw[:, lblk:lblk + 1], start=(lblk == 0),
                             stop=(lblk == 2))
        z = cp.tile([33, 1], f32)
        nc.scalar.activation(z[:], zp[:], AF.Tanh, bias=linb)
        h1p = pp.tile([48, 1], f32, space="PSUM", tag="psB")
        nc.tensor.matmul(h1p[:], lhsT=m1w, rhs=z[:], start=True, stop=True)
        h1 = cp.tile([48, 1], f32)
        nc.scalar.activation(h1[:], h1p[:], AF.Tanh, bias=m1b)
        h2p = pp.tile([16, 1], f32, space="PSUM", tag="psB")
        nc.tensor.matmul(h2p[:], lhsT=m2w, rhs=h1[:], start=True,
                         stop=True)
        h2 = cp.tile([16, 1], f32)
        nc.scalar.activation(h2[:], h2p[:], AF.Tanh, bias=m2b)
        h3p = pp.tile([1, 1], f32, space="PSUM", tag="psB")
        nc.tensor.matmul(h3p[:], lhsT=m3w, rhs=h2[:], start=True,
                         stop=True)
        h3 = cp.tile([1, 1], f32)
        nc.scalar.activation(h3[:], h3p[:], AF.Sigmoid, bias=m3b)
        nc.scalar.activation(h3[:], h3[:], AF.Sigmoid)
        nc.sync.dma_start(out.ap(), h3[:])
        if dbg:
            nc.gpsimd.dma_start(dbg['d_ro'].ap()[1:33, :], roall[:])
            nc.gpsimd.dma_start(
                dbg['d_ro'].ap()[0:1, :].rearrange("o (l p) -> p (o l)",
                                                   p=128), ro1s[:])

        for pool in (s2g, s2pool, dp, pp, wp, big, cp):
            pool.release()

    nc.compile()
    return nc


# ------------------------------------------------------------------- driver
def kernel(**inputs):
    per_core, meta = host_prep(inputs)
    key = meta
    if key not in _build_cache:
        _build_cache[key] = build_nc(*meta, debug=bool(
            int(__import__('os').environ.get('DMOI_DEBUG', '0'))))
    nc = _build_cache[key]
    import os as _os
    want_trace = bool(int(_os.environ.get('DMOI_TRACE', '0')))
    try:
        res = bass_utils.run_bass_kernel_spmd(
            nc, per_core, core_ids=list(range(NC)), trace=want_trace)
    except Exception:
        if not want_trace:
            raise
        res = bass_utils.run_bass_kernel_spmd(
            nc, per_core, core_ids=list(range(NC)))
    r0 = res.results[0]
    kernel.last_results = res
    return r0['out'].astype(np.float32)


# revision 7
# speedup vs baseline: 4.9092x; 1.3036x over previous
"""DeepMOI GNN kernel for 8 Trainium2 NeuronCores (Bass/Tile).

Sharding: stage-1 full-graph SAGE aggregation is sharded by dst-node slice
(1024 nodes/core) with an AllGather of the aggregated means; the dense
per-node compute is replicated. Stage-2 pathway subgraphs are data-parallel:
4 graphs per core, processed sequentially. readout2 rows are AllGathered
and the tiny final MLP is computed on every core.

Aggregation strategy: edges sorted by dst, packed into 128-dst windows;
segment-sum is a one-hot matmul on PE (one-hot built in one broadcast DVE
compare from static dst-local columns). Messages are bulk-gathered from a
row table in DRAM with dma_gather; the row tables are produced with XBAR
DMA transposes (no PE/scalar involvement). Global-attention readouts are
computed row-wise (gate.x via feature-contraction matmuls, then a
broadcast-weighted reduction) so no node-major transpose of x is needed.
Top-k masks use a 16-probe multisection search.

Host->device traffic is minimized: per-core-identical constants (weights,
h) ship sharded 1/8th per core and are AllGathered on device; gather
indices ship unreplicated ([16,X] int16, fanned out to 128 partitions on
device); reciprocal degrees are computed on device from the one-hot tiles;
iota/identity/window masks are generated on device. The jax persistent
compilation cache is enabled so repeat launches skip XLA re-compilation.
"""
import sys
sys.path.insert(0, '/opt/trn_rl_repo')
import numpy as np
import ml_dtypes

import jax
try:
    jax.config.update("jax_compilation_cache_dir", "/tmp/.dmoi_jax_cache")
    jax.config.update("jax_persistent_cache_min_entry_size_bytes", -1)
    jax.config.update("jax_persistent_cache_min_compile_time_secs", 0)
except Exception:
    pass

import concourse.bass as bass
import concourse.bacc as bacc
import concourse.tile as tile
import concourse.mybir as mybir
from concourse import bass_utils

N = 8000
NP = 8192
P = 32
ES = 4096
DIN = 16
D = 128
NC = 8
GPC = P // NC
W2 = 64
S1SLICE = NP // NC
S1W = S1SLICE // 128
KS = (6400, 5120, 4096)
NPROBE = 16
NITER = 4

AF = mybir.ActivationFunctionType
OP = mybir.AluOpType
dt = mybir.dt
BF = ml_dtypes.bfloat16
AX = mybir.AxisListType

_build_cache = {}

# Fb blob column map (f32 [128, FBW])
FB_BLA = 0          # [128, 1]
FB_BLS = 1          # [128, 5]
FB_BREL = 6         # [128, 3]
FB_NW = 9
FB_NB = 10
FB_NMS = 11
FB_LINW = 12        # [128, 3]
FB_M1W = 15         # [33, 48]
FB_M1B = 63         # [48, 1]
FB_M2W = 64         # [48, 16]
FB_M2B = 80         # [16, 1]
FB_M3W = 81         # [16, 1]
FB_M3B = 82         # [1, 1]
FB_LINB = 83        # [33, 1]
FBW = 84
# Wb blob column map (bf16 [128, WBW])
WB_WLS = 0          # [128, 5*128]
WB_WRS = 640        # [128, 5*128]
WB_GATE = 1280      # [128, 6]
WB_WREL = 1286      # [128, 3]
WB_WROOT = 1289     # [128, 3]
WBW = 1292


# ----------------------------------------------------------------- host prep
def wrap16(idx):
    n = idx.shape[0]
    return np.ascontiguousarray(idx.reshape(n // 16, 16).T.astype(np.int16))


def window_major_edges(src, dst, nwin, cmax, sentinel):
    order = np.argsort(dst, kind='stable')
    src, dst = src[order], dst[order]
    nslot = nwin * cmax * 128
    sp = np.full(nslot, sentinel, np.int64)
    dl = np.full((128, nwin * cmax), -1.0, np.float32)
    for w in range(nwin):
        lo = np.searchsorted(dst, w * 128)
        hi = np.searchsorted(dst, (w + 1) * 128)
        if hi <= lo:
            continue
        assert hi - lo <= cmax * 128, f"window {w}: {hi-lo}"
        base = w * cmax * 128
        sp[base:base + (hi - lo)] = src[lo:hi]
        for k in range(cmax):
            a = lo + k * 128
            if a >= hi:
                break
            b = min(hi, a + 128)
            dl[:b - a, w * cmax + k] = (dst[a:b] - w * 128).astype(np.float32)
    return sp, dl


def host_prep(inputs):
    h = np.asarray(inputs['h'], np.float32)
    ei = np.asarray(inputs['edge_index'], np.int64)
    sei = np.asarray(inputs['sub_edge_index'], np.int64)

    src1 = np.concatenate([ei[0], np.arange(N)])
    dst1 = np.concatenate([ei[1], np.arange(N)])
    cnt_w = np.bincount(dst1 // 128, minlength=64)
    CMAX1 = int(np.ceil(cnt_w.max() / 128))

    s1 = []
    for c in range(NC):
        m = (dst1 >= c * S1SLICE) & (dst1 < (c + 1) * S1SLICE)
        sp, dl = window_major_edges(src1[m], dst1[m] - c * S1SLICE,
                                    S1W, CMAX1, NP - 1)
        s1.append((wrap16(sp), dl.astype(BF)))

    hT = np.zeros((16, NP), BF)
    hT[:, :N] = h.T.astype(BF)

    ssrc = (sei[0].reshape(P, ES) - (np.arange(P) * N)[:, None])
    sdst = (sei[1].reshape(P, ES) - (np.arange(P) * N)[:, None])
    CMAX2 = 1
    for g in range(P):
        CMAX2 = max(CMAX2, int(np.ceil(
            np.bincount(sdst[g] // 128, minlength=W2).max() / 128)))

    s2src, s2dl = [], []
    for c in range(NC):
        a_s, a_d = [], []
        for j in range(GPC):
            g = c * GPC + j
            sp, dl = window_major_edges(ssrc[g], sdst[g], W2, CMAX2, NP - 1)
            a_s.append(wrap16(sp))
            a_d.append(dl)
        s2src.append(np.ascontiguousarray(np.concatenate(a_s, axis=1)))
        s2dl.append(np.ascontiguousarray(
            np.concatenate(a_d, axis=1).astype(BF)))

    W = {k: np.asarray(inputs[k], np.float32) for k in inputs}

    Fb = np.zeros((128, FBW), np.float32)
    Fb[:, FB_BLA] = W['bl_a']
    Fb[:, FB_BLS:FB_BLS + 5] = W['bl_s'].T
    Fb[:, FB_BREL:FB_BREL + 3] = np.tile(W['pool_brel'][None, :], (128, 1))
    Fb[:, FB_NW] = W['norm_w']
    Fb[:, FB_NB] = W['norm_b']
    Fb[:, FB_NMS] = W['norm_ms']
    Fb[:, FB_LINW:FB_LINW + 3] = W['lin_w'].reshape(3, 128).T
    Fb[0:33, FB_M1W:FB_M1W + 48] = W['m1_w']
    Fb[0:48, FB_M1B] = W['m1_b']
    Fb[0:48, FB_M2W:FB_M2W + 16] = W['m2_w']
    Fb[0:16, FB_M2B] = W['m2_b']
    Fb[0:16, FB_M3W] = W['m3_w'][:, 0]
    Fb[0, FB_M3B] = W['m3_b'][0]
    Fb[0:33, FB_LINB] = W['lin_b'][0]

    Wb = np.zeros((128, WBW), BF)
    Wb[:, WB_WLS:WB_WLS + 640] = np.ascontiguousarray(
        W['Wl_s'].transpose(1, 0, 2)).reshape(128, 640).astype(BF)
    Wb[:, WB_WRS:WB_WRS + 640] = np.ascontiguousarray(
        W['Wr_s'].transpose(1, 0, 2)).reshape(128, 640).astype(BF)
    Wb[:, WB_GATE:WB_GATE + 6] = W['gate_w'].T.astype(BF)
    Wb[:, WB_WREL:WB_WREL + 3] = W['pool_wrel'].T.astype(BF)
    Wb[:, WB_WROOT:WB_WROOT + 3] = W['pool_wroot'].T.astype(BF)

    WA = np.zeros((16, 256), BF)
    WA[:, 0:128] = W['Wl_a'].astype(BF)
    WA[:, 128:256] = W['Wr_a'].astype(BF)

    per_core = []
    for c in range(NC):
        m = dict(
            Wb_sh=np.ascontiguousarray(Wb[16 * c:16 * (c + 1), :]),
            Fb_sh=np.ascontiguousarray(Fb[16 * c:16 * (c + 1), :]),
            hT_sh=np.ascontiguousarray(hT[2 * c:2 * (c + 1), :]),
            WA_sh=np.ascontiguousarray(WA[2 * c:2 * (c + 1), :]),
        )
        m['s1_src'], m['s1_dl'] = s1[c]
        m['s2_src'], m['s2_dl'] = s2src[c], s2dl[c]
        per_core.append(m)
    return per_core, (CMAX1, CMAX2)


# ---------------------------------------------------------------- the kernel
def build_nc(CMAX1, CMAX2, debug=False):
    NP1 = S1W * CMAX1 * 128
    NP2 = W2 * CMAX2 * 128
    PR1 = S1W * CMAX1
    PR2 = W2 * CMAX2

    nc = bacc.Bacc("TRN2", target_bir_lowering=False, debug=False,
                   num_devices=NC)
    f32, bf16 = dt.float32, dt.bfloat16

    def inp(name, shape, d=f32):
        return nc.dram_tensor(name, shape, d, kind="ExternalInput")

    t_hT = inp('hT_sh', [2, NP], bf16)
    t_s1src = inp('s1_src', [16, NP1 // 16], dt.int16)
    t_s1dl = inp('s1_dl', [128, PR1], bf16)
    t_s2src = inp('s2_src', [16, GPC * NP2 // 16], dt.int16)
    t_s2dl = inp('s2_dl', [128, GPC * PR2], bf16)
    t_Wb = inp('Wb_sh', [16, WBW], bf16)
    t_WA = inp('WA_sh', [2, 256], bf16)
    t_Fb = inp('Fb_sh', [16, FBW])

    out = nc.dram_tensor('out', [1, 1], f32, kind="ExternalOutput")
    dbg = {}
    if debug:
        for nm, shp in (('d_x1T', [128, NP]), ('d_ro', [33, 384]),
                        ('d_sc0', [128, W2]), ('d_na0', [128, W2])):
            dbg[nm] = nc.dram_tensor(nm, shp, f32, kind="ExternalOutput")

    with tile.TileContext(nc, trace_sim=False) as tc:
        cp = tc.alloc_tile_pool(name="const", bufs=1)
        big = tc.alloc_tile_pool(name="big", bufs=1)
        wp = tc.alloc_tile_pool(name="wk", bufs=2)
        pp = tc.alloc_tile_pool(name="ps", bufs=2, space="PSUM")
        dp = tc.alloc_tile_pool(name="dram", bufs=1, space="DRAM")
        grp = [list(range(NC))]

        # ------------- packed constants (sharded upload + device AllGather).
        # Collectives cannot read IO tensors; bounce shards via internal DRAM.
        def gather_shared(t_sh, shard_shape, d):
            stage = dp.tile(shard_shape, d)
            nc.sync.dma_start(stage[:], t_sh.ap())
            full = dp.tile([shard_shape[0] * NC, shard_shape[1]], d)
            nc.gpsimd.collective_compute("AllGather", OP.bypass,
                                         replica_groups=grp,
                                         ins=[stage.opt()],
                                         outs=[full.opt()])
            return full

        Wb_full = gather_shared(t_Wb, [16, WBW], bf16)
        Wb = cp.tile([128, WBW], bf16, tag="Wb")
        nc.sync.dma_start(Wb[:], Wb_full[:])
        Fb_full = gather_shared(t_Fb, [16, FBW], f32)
        Fb = cp.tile([128, FBW], f32, tag="Fb")
        nc.sync.dma_start(Fb[:], Fb_full[:])
        WA_full = gather_shared(t_WA, [2, 256], bf16)
        WA = cp.tile([16, 256], bf16, tag="WA")
        nc.sync.dma_start(WA[:], WA_full[:])
        hT_full = gather_shared(t_hT, [2, NP], bf16)

        def WlS(i):
            return Wb[:, WB_WLS + i * 128:WB_WLS + (i + 1) * 128]

        def WrS(i):
            return Wb[:, WB_WRS + i * 128:WB_WRS + (i + 1) * 128]

        gate = Wb[:, WB_GATE:WB_GATE + 6]
        wrel = Wb[:, WB_WREL:WB_WREL + 3]
        wroot = Wb[:, WB_WROOT:WB_WROOT + 3]
        WlA = WA[:, 0:128]
        WrA = WA[:, 128:256]
        blA = Fb[:, FB_BLA:FB_BLA + 1]
        blS = Fb[:, FB_BLS:FB_BLS + 5]
        brel = Fb[:, FB_BREL:FB_BREL + 3]
        nw = Fb[:, FB_NW:FB_NW + 1]
        nb = Fb[:, FB_NB:FB_NB + 1]
        nms = Fb[:, FB_NMS:FB_NMS + 1]
        linw = Fb[:, FB_LINW:FB_LINW + 3]
        m1w = Fb[0:33, FB_M1W:FB_M1W + 48]
        m1b = Fb[0:48, FB_M1B:FB_M1B + 1]
        m2w = Fb[0:48, FB_M2W:FB_M2W + 16]
        m2b = Fb[0:16, FB_M2B:FB_M2B + 1]
        m3w = Fb[0:16, FB_M3W:FB_M3W + 1]
        m3b = Fb[0:1, FB_M3B:FB_M3B + 1]
        linb = Fb[0:33, FB_LINB:FB_LINB + 1]

        # ------------- generated constants
        iota = cp.tile([128, 128], f32)
        nc.gpsimd.iota(iota[:], pattern=[[1, 128]], base=0,
                       channel_multiplier=0,
                       allow_small_or_imprecise_dtypes=True)
        ones128 = cp.tile([128, 128], f32)
        nc.vector.memset(ones128[:], 1.0)
        identf = cp.tile([128, 128], f32)
        nc.gpsimd.affine_select(identf[:], ones128[:], pattern=[[-1, 128]],
                                compare_op=OP.is_equal, fill=0.0, base=0,
                                channel_multiplier=1)
        nid = cp.tile([128, W2], f32)
        nc.gpsimd.iota(nid[:], pattern=[[128, W2]], base=0,
                       channel_multiplier=1,
                       allow_small_or_imprecise_dtypes=True)
        na0c = cp.tile([128, W2], f32)
        nc.vector.tensor_scalar(out=na0c[:], in0=nid[:],
                                scalar1=float(N) - 0.5, scalar2=None,
                                op0=OP.is_ge)
        nc.vector.tensor_scalar(out=na0c[:], in0=na0c[:], scalar1=-1.0,
                                scalar2=1.0, op0=OP.mult, op1=OP.add)
        probei = cp.tile([1, NPROBE], f32)
        nc.gpsimd.iota(probei[:], pattern=[[1, NPROBE]], base=1,
                       channel_multiplier=0,
                       allow_small_or_imprecise_dtypes=True)
        ones_c = cp.tile([128, 1], f32); nc.vector.memset(ones_c[:], 1.0)
        ones_cb = cp.tile([128, 1], bf16); nc.vector.memset(ones_cb[:], 1.0)
        ones_r = cp.tile([1, 128], f32); nc.vector.memset(ones_r[:], 1.0)
        ones_rb = cp.tile([1, 128], bf16); nc.vector.memset(ones_rb[:], 1.0)

        xT = big.tile([128, NP], bf16, tag="xT")       # current features^T
        rows = big.tile([128, W2, 128], bf16, tag="rows")
        ro2s = cp.tile([128, 3, GPC], f32)
        ro1s = cp.tile([128, 3], f32)
        xrows1 = dp.tile([NP, 128], bf16)              # stage-1 row table
        xrows2 = dp.tile([NP, 128], bf16)              # stage-2 scratch table
        utab = dp.tile([NP, 128], bf16)                # u table (DRAM)
        xTd = dp.tile([128, NP], bf16)                 # transpose staging

        # =========================================================== helpers
        def transpose_pass(srcT, dst):
            """srcT [128, NP] bf16 -> row table dst [NP, 128] via XBAR DMA
            transposes (bounced through DRAM; no PE/scalar involvement)."""
            nc.sync.dma_start(xTd[:], srcT[:])
            for w in range(W2):
                nc.sync.dma_start_transpose(rows[:, w, :],
                                            xTd[:, w * 128:(w + 1) * 128])
            nc.sync.dma_start(
                dst[:].rearrange("(w p) e -> p w e", p=128), rows[:])

        rowbounce = dp.tile([1, NP], bf16)
        frow = cp.tile([1, NP], bf16)
        abrow = cp.tile([1, NP], bf16)

        def row_from_grid(gtile, rowt, wn):
            nc.gpsimd.dma_start(
                rowbounce[0:1, 0:wn * 128].rearrange("o (w p) -> o p w",
                                                     p=128), gtile[:])
            nc.sync.dma_start(rowt[0:1, 0:wn * 128],
                              rowbounce[0:1, 0:wn * 128])

        # valid-node row mask [1, NP] (1 for node < N)
        narow = cp.tile([1, NP], bf16)
        row_from_grid(na0c, narow, W2)

        def pe_bcast_col(val11):
            """[1,1] f32 -> psum [128,1]."""
            ps = pp.tile([128, 1], f32, space="PSUM", tag="psB")
            nc.tensor.matmul(ps[:], lhsT=ones_r[:], rhs=val11[:],
                             start=True, stop=True)
            return ps

        def weighted_readout(xsrcT, ab_row, slot):
            """slot[d] = sum_n ab_row[n] * xsrcT[d, n]."""
            parts = wp.tile([128, NP // 512], f32, tag="parts")
            for t in range(NP // 512):
                sl = slice(t * 512, (t + 1) * 512)
                ob = pp.tile([128, 512], f32, space="PSUM", tag="psA")
                nc.tensor.matmul(ob[:], lhsT=ones_rb[:],
                                 rhs=ab_row[0:1, sl], start=True, stop=True)
                tmp = wp.tile([128, 512], f32, tag="tmpws")
                nc.vector.tensor_tensor(out=tmp[:], in0=ob[:],
                                        in1=xsrcT[:, sl], op=OP.mult)
                nc.vector.tensor_reduce(out=parts[:, t:t + 1], in_=tmp[:],
                                        axis=AX.X, op=OP.add)
            nc.vector.tensor_reduce(out=slot, in_=parts[:], axis=AX.X,
                                    op=OP.add)

        def readout1(layer):
            """stage-1 global attention over xT, row-wise."""
            sums = wp.tile([1, NP // 512], f32, tag="sums")
            for t in range(NP // 512):
                sl = slice(t * 512, (t + 1) * 512)
                gp = pp.tile([1, 512], f32, space="PSUM", tag="psB")
                nc.tensor.matmul(gp[:], lhsT=gate[:, layer:layer + 1],
                                 rhs=xT[:, sl], start=True, stop=True)
                exr = wp.tile([1, 512], f32, tag="exr")
                nc.scalar.activation(exr[:], gp[:], AF.Exp)
                nc.vector.tensor_tensor(out=abrow[0:1, sl], in0=exr[:],
                                        in1=narow[0:1, sl], op=OP.mult)
                nc.vector.tensor_reduce(out=sums[:, t:t + 1],
                                        in_=abrow[0:1, sl], axis=AX.X,
                                        op=OP.add)
            tot = wp.tile([1, 1], f32, tag="tot1")
            nc.vector.tensor_reduce(out=tot[:], in_=sums[:], axis=AX.X,
                                    op=OP.add)
            rden = wp.tile([1, 1], f32, tag="rden")
            nc.vector.reciprocal(rden[:], tot[:])
            nc.vector.tensor_scalar(out=abrow[:], in0=abrow[:],
                                    scalar1=rden[:, 0:1], scalar2=None,
                                    op0=OP.mult)
            weighted_readout(xT, abrow, ro1s[:, layer:layer + 1])

        # ====================================================== stage 1
        s1pool = tc.alloc_tile_pool(name="s1", bufs=1)
        s1src = s1pool.tile([128, NP1 // 16], dt.int16, tag="s1src")
        for k in range(8):
            nc.sync.dma_start(s1src[16 * k:16 * (k + 1), :], t_s1src.ap())
        s1dlb = s1pool.tile([128, PR1], bf16, tag="s1dlb")
        nc.sync.dma_start(s1dlb[:], t_s1dl.ap())
        s1dl = s1pool.tile([128, PR1], f32, tag="s1dl")
        nc.vector.tensor_copy(out=s1dl[:], in_=s1dlb[:])
        eq1 = s1pool.tile([128, PR1, 128], bf16)
        nc.vector.tensor_tensor(
            out=eq1[:],
            in0=iota[:].rearrange("p (o e) -> p o e", o=1)
                .to_broadcast([128, PR1, 128]),
            in1=s1dl[:].rearrange("p (c o) -> p c o", o=1)
                .to_broadcast([128, PR1, 128]),
            op=OP.is_equal)

        # reciprocal in-degree of this core's dst slice (same for all layers)
        degp1 = pp.tile([128, S1W], f32, space="PSUM", tag="psB")
        for w in range(S1W):
            for k in range(CMAX1):
                pr = w * CMAX1 + k
                nc.tensor.matmul(degp1[:, w:w + 1], lhsT=eq1[:, pr, :],
                                 rhs=ones_cb[:], start=(k == 0),
                                 stop=(k == CMAX1 - 1))
        rdeg1g = wp.tile([128, S1W], f32, tag="rdeg1g")
        nc.vector.tensor_scalar(out=rdeg1g[:], in0=degp1[:], scalar1=1.0,
                                scalar2=None, op0=OP.max)
        nc.vector.reciprocal(rdeg1g[:], rdeg1g[:])
        row_from_grid(rdeg1g, frow, S1W)
        rdegb = s1pool.tile([128, S1SLICE], bf16, tag="rdegb")
        for t in range(S1SLICE // 512):
            sl = slice(t * 512, (t + 1) * 512)
            ob = pp.tile([128, 512], f32, space="PSUM", tag="psA")
            nc.tensor.matmul(ob[:], lhsT=ones_rb[:], rhs=frow[0:1, sl],
                             start=True, stop=True)
            nc.scalar.copy(out=rdegb[:, sl], in_=ob[:])

        # publish h rows (feature-padded with zeros) for the layer-0 gather
        nc.vector.memset(xT[:], 0.0)
        nc.sync.dma_start(xT[0:16, :], hT_full[:])
        transpose_pass(xT, xrows1)

        agb_in = dp.tile([128, S1SLICE], bf16)
        agb_out = dp.tile([128 * NC, S1SLICE], bf16)
        meanT = big.tile([128, NP], bf16, tag="meanT")

        def s1_layer(layer):
            srcT = xT
            aggp = pp.tile([128, S1W, 128], f32, space="PSUM", tag="pagg")
            for half in range(2):
                msgs = s1pool.tile([128, PR1 // 2, 128], bf16, tag="msgs1")
                nc.gpsimd.dma_gather(
                    out_ap=msgs[:], in_ap=xrows1[:],
                    idxs_ap=s1src[:, half * (NP1 // 32):(half + 1) * (NP1 // 32)],
                    num_idxs=NP1 // 2, num_idxs_reg=NP1 // 2, elem_size=128,
                    single_packet=False)
                for w in range(S1W // 2 * half, S1W // 2 * (half + 1)):
                    for k in range(CMAX1):
                        pr = w * CMAX1 + k
                        lpr = pr - half * (PR1 // 2)
                        nc.tensor.matmul(aggp[:, w, :], lhsT=msgs[:, lpr, :],
                                         rhs=eq1[:, pr, :], start=(k == 0),
                                         stop=(k == CMAX1 - 1))
            mslice = wp.tile([128, S1SLICE], bf16, tag="mslice")
            nc.vector.tensor_tensor(
                out=mslice[:], in0=aggp[:].rearrange("p w e -> p (w e)"),
                in1=rdegb[:], op=OP.mult)
            nc.sync.dma_start(agb_in[:], mslice[:])
            nc.gpsimd.collective_compute(
                "AllGather", OP.bypass, replica_groups=grp,
                ins=[agb_in.opt()], outs=[agb_out.opt()])
            # reassemble meanT full [128, NP]
            nc.sync.dma_start(
                meanT[:].rearrange("p (c s) -> p c s", c=NC),
                agb_out[:].rearrange("(c p) s -> p c s", p=128))
            Wl = WlA if layer == 0 else WlS(layer - 1)
            Wr = WrA if layer == 0 else WrS(layer - 1)
            bl = blA if layer == 0 else blS[:, layer - 1:layer]
            kdim = 16 if layer == 0 else 128
            for t in range(NP // 512):
                sl = slice(t * 512, (t + 1) * 512)
                xp = pp.tile([128, 512], f32, space="PSUM", tag="psA")
                nc.tensor.matmul(xp[:], lhsT=Wl[0:kdim, :],
                                 rhs=meanT[0:kdim, sl], start=True, stop=False)
                nc.tensor.matmul(xp[:], lhsT=Wr[0:kdim, :],
                                 rhs=srcT[0:kdim, sl], start=False, stop=True)
                nc.scalar.activation(xT[:, sl], xp[:], AF.Tanh, bias=bl)

        def gnorm():
            mu = wp.tile([128, 1], f32, tag="mu")
            nc.vector.tensor_reduce(out=mu[:], in_=xT[:, 0:N], axis=AX.X,
                                    op=OP.add)
            nc.vector.tensor_scalar(out=mu[:], in0=mu[:], scalar1=1.0 / N,
                                    scalar2=None, op0=OP.mult)
            mums = wp.tile([128, 1], f32, tag="mums")
            nc.vector.tensor_tensor(out=mums[:], in0=mu[:], in1=nms[:],
                                    op=OP.mult)
            o = meanT  # meanT is free between layers; reuse as scratch
            nc.vector.tensor_scalar(out=o[:], in0=xT[:], scalar1=mums[:],
                                    scalar2=None, op0=OP.subtract)
            var = wp.tile([128, 1], f32, tag="var")
            sq = rows[:].rearrange("p w e -> p (w e)")  # rows free here too
            nc.vector.tensor_tensor(out=sq, in0=o[:], in1=o[:], op=OP.mult)
            nc.vector.tensor_reduce(out=var[:], in_=sq[:, 0:N], axis=AX.X,
                                    op=OP.add)
            nc.vector.tensor_scalar(out=var[:], in0=var[:], scalar1=1.0 / N,
                                    scalar2=1e-5, op0=OP.mult, op1=OP.add)
            rstd = wp.tile([128, 1], f32, tag="rstd")
            nc.vector.reciprocal(rstd[:], var[:])
            nc.scalar.activation(rstd[:], rstd[:], AF.Sqrt)
            sc = wp.tile([128, 1], f32, tag="scn")
            nc.vector.tensor_tensor(out=sc[:], in0=rstd[:], in1=nw[:],
                                    op=OP.mult)
            nc.vector.tensor_scalar(out=xT[:], in0=o[:], scalar1=sc[:],
                                    scalar2=nb[:], op0=OP.mult, op1=OP.add)

        for layer in range(3):
            s1_layer(layer)
            readout1(layer)
            if layer < 2:
                gnorm()
            transpose_pass(xT, xrows1)

        if dbg:
            nc.gpsimd.dma_start(dbg['d_x1T'].ap(), xT[:])

        s1pool.release()

        # ====================================================== stage 2
        s2pool = tc.alloc_tile_pool(name="s2", bufs=1)
        s2g = tc.alloc_tile_pool(name="s2g", bufs=1)
        s2src = s2pool.tile([128, GPC * NP2 // 16], dt.int16, tag="s2src")
        for k in range(8):
            nc.sync.dma_start(s2src[16 * k:16 * (k + 1), :], t_s2src.ap())
        s2dlb = s2pool.tile([128, GPC * PR2], bf16, tag="s2dlb")
        nc.sync.dma_start(s2dlb[:], t_s2dl.ap())
        s2dl = s2pool.tile([128, GPC * PR2], f32, tag="s2dl")
        nc.vector.tensor_copy(out=s2dl[:], in_=s2dlb[:])

        xgT = s2pool.tile([128, NP], bf16, tag="xgT")
        eq2 = s2pool.tile([128, PR2, 128], bf16, tag="eq2")
        rbc = s2pool.tile([128, NP], bf16, tag="rbc")
        ubounce = dp.tile([3, NP], bf16)

        def ugT(sl):
            # u/uroot/gx rows live in partitions 0:3 of the `rows` staging
            # tile — temporally disjoint from its transpose-staging use.
            return rows[:].rearrange("p w e -> p (w e)")[0:3, sl]
        ones_pr = s2pool.tile([128, PR2], bf16, tag="ones_pr")
        nc.vector.memset(ones_pr[:], 1.0)

        for j in range(GPC):
            # per-graph eq tiles (unscaled one-hot), one broadcast compare
            nc.vector.tensor_tensor(
                out=eq2[:],
                in0=iota[:].rearrange("p (o e) -> p o e", o=1)
                    .to_broadcast([128, PR2, 128]),
                in1=s2dl[:, j * PR2:(j + 1) * PR2]
                    .rearrange("p (c o) -> p c o", o=1)
                    .to_broadcast([128, PR2, 128]),
                op=OP.is_equal)
            nc.vector.tensor_copy(out=xgT[:], in_=xT[:])
            na_g = cp.tile([128, W2], f32, tag=f"nag{j}")
            f_g = cp.tile([128, W2], f32, tag=f"fg{j}")
            nc.vector.tensor_copy(out=na_g[:], in_=na0c[:])

            for l in range(3):
                li = 2 + l
                # ---- messages gather (stage-1 table at l=0, else own)
                msgs = s2g.tile([128, PR2, 128], bf16, tag="m2")
                nc.gpsimd.dma_gather(
                    out_ap=msgs[:], in_ap=(xrows1[:] if l == 0 else xrows2[:]),
                    idxs_ap=s2src[:, j * (NP2 // 16):(j + 1) * (NP2 // 16)],
                    num_idxs=NP2, num_idxs_reg=NP2, elem_size=128,
                    single_packet=False)
                # ---- reciprocal alive-in-degree -> rbc [128, NP]
                if l == 0:
                    ind = ones_pr          # all edges live before 1st pool
                else:
                    ind = wp.tile([128, PR2], bf16, tag="ind")
                    nc.vector.tensor_scalar(out=ind[:],
                                            in0=msgs[:, :, 0:1].rearrange(
                                                "p c o -> p (c o)"),
                                            scalar1=0.0, scalar2=None,
                                            op0=OP.not_equal)
                degp = pp.tile([128, W2], f32, space="PSUM", tag="psB")
                for w in range(W2):
                    for k in range(CMAX2):
                        pr = w * CMAX2 + k
                        nc.tensor.matmul(
                            degp[:, w:w + 1], lhsT=eq2[:, pr, :],
                            rhs=ind[:, pr:pr + 1], start=(k == 0),
                            stop=(k == CMAX2 - 1))
                rdeg = wp.tile([128, W2], f32, tag="rdeg")
                nc.vector.tensor_scalar(out=rdeg[:], in0=degp[:],
                                        scalar1=1.0, scalar2=None,
                                        op0=OP.max)
                nc.vector.reciprocal(rdeg[:], rdeg[:])
                nc.vector.tensor_tensor(out=rdeg[:], in0=rdeg[:],
                                        in1=na_g[:], op=OP.mult)
                row_from_grid(rdeg, frow, W2)
                for t in range(NP // 512):
                    sl = slice(t * 512, (t + 1) * 512)
                    ob = pp.tile([128, 512], f32, space="PSUM", tag="psA")
                    nc.tensor.matmul(ob[:], lhsT=ones_rb[:],
                                     rhs=frow[0:1, sl], start=True,
                                     stop=True)
                    nc.scalar.copy(out=rbc[:, sl], in_=ob[:])
                # ---- aggregation matmuls + fused mean scale
                for grpi in range(8):
                    agp = pp.tile([128, 8, 128], f32, space="PSUM",
                                  tag="pagg")
                    for wi in range(8):
                        w = grpi * 8 + wi
                        for k in range(CMAX2):
                            pr = w * CMAX2 + k
                            nc.tensor.matmul(agp[:, wi, :],
                                             lhsT=msgs[:, pr, :],
                                             rhs=eq2[:, pr, :],
                                             start=(k == 0),
                                             stop=(k == CMAX2 - 1))
                    sl = slice(grpi * 1024, (grpi + 1) * 1024)
                    nc.vector.tensor_tensor(
                        out=meanT[:, sl],
                        in0=agp[:].rearrange("p w e -> p (w e)"),
                        in1=rbc[:, sl], op=OP.mult)
                # ---- x' = tanh(Wl.T meanT + Wr.T xm + bl)
                for t in range(NP // 512):
                    sl = slice(t * 512, (t + 1) * 512)
                    xp = pp.tile([128, 512], f32, space="PSUM", tag="psA")
                    nc.tensor.matmul(xp[:], lhsT=WlS(li), rhs=meanT[:, sl],
                                     start=True, stop=False)
                    nc.tensor.matmul(xp[:], lhsT=WrS(li), rhs=xgT[:, sl],
                                     start=False, stop=True)
                    nc.scalar.activation(xgT[:, sl], xp[:], AF.Tanh,
                                         bias=blS[:, li:li + 1])
                # ---- u/uroot/gx rows via feature-contraction matmuls
                wcols = cp.tile([128, 3], bf16, tag=f"wcols{j}_{l}")
                nc.vector.tensor_copy(out=wcols[:, 0:1], in_=wrel[:, l:l + 1])
                nc.vector.tensor_copy(out=wcols[:, 1:2],
                                      in_=wroot[:, l:l + 1])
                nc.vector.tensor_copy(out=wcols[:, 2:3],
                                      in_=gate[:, 3 + l:4 + l])
                for t in range(NP // 512):
                    sl = slice(t * 512, (t + 1) * 512)
                    up = pp.tile([3, 512], f32, space="PSUM", tag="psB")
                    nc.tensor.matmul(up[:], lhsT=wcols[:], rhs=xgT[:, sl],
                                     start=True, stop=True)
                    nc.scalar.copy(out=ugT(sl), in_=up[:])
                # u table rows + uroot/gx grids (bounced through DRAM)
                nc.sync.dma_start(ubounce[:], ugT(slice(None)))
                nc.gpsimd.dma_start(utab[:, 0:1],
                                    ubounce[0:1, :].rearrange("o n -> n o"))
                urootg = wp.tile([128, W2], bf16, tag="urootg")
                nc.sync.dma_start(
                    urootg[:],
                    ubounce[1:2, :].rearrange("o (w p) -> p (o w)", p=128))
                gxg_raw = wp.tile([128, W2], bf16, tag="gxgr")
                nc.sync.dma_start(
                    gxg_raw[:],
                    ubounce[2:3, :].rearrange("o (w p) -> p (o w)", p=128))
                # ---- score pass
                umsg = s2g.tile([128, PR2, 128], bf16, tag="m2")
                nc.gpsimd.dma_gather(
                    out_ap=umsg[:], in_ap=utab[:],
                    idxs_ap=s2src[:, j * (NP2 // 16):(j + 1) * (NP2 // 16)],
                    num_idxs=NP2, num_idxs_reg=NP2, elem_size=128,
                    single_packet=False)
                scp = pp.tile([128, W2], f32, space="PSUM", tag="psB")
                for w in range(W2):
                    for k in range(CMAX2):
                        pr = w * CMAX2 + k
                        nc.tensor.matmul(scp[:, w:w + 1], lhsT=eq2[:, pr, :],
                                         rhs=umsg[:, pr, 0:1],
                                         start=(k == 0),
                                         stop=(k == CMAX2 - 1))
                score = cp.tile([128, W2], f32, tag=f"score{j}_{l}")
                nc.vector.tensor_tensor(out=score[:], in0=scp[:],
                                        in1=urootg[:], op=OP.add)
                nc.vector.tensor_scalar(out=score[:], in0=score[:],
                                        scalar1=brel[:, l:l + 1],
                                        scalar2=None, op0=OP.add)
                if dbg and j == 0 and l == 0:
                    nc.gpsimd.dma_start(dbg['d_sc0'].ap(), score[:])
                # ---- top-k threshold via multiprobe
                sm_ = cp.tile([128, W2], f32, tag=f"smask{j}_{l}")
                nc.vector.tensor_tensor(out=sm_[:], in0=score[:], in1=na_g[:],
                                        op=OP.mult)
                pen = wp.tile([128, W2], f32, tag="pen")
                nc.vector.tensor_scalar(out=pen[:], in0=na_g[:],
                                        scalar1=1e6, scalar2=-1e6,
                                        op0=OP.mult, op1=OP.add)
                nc.vector.tensor_tensor(out=sm_[:], in0=sm_[:], in1=pen[:],
                                        op=OP.add)
                lo = cp.tile([1, 1], f32, tag=f"lo{j}_{l}")
                st = cp.tile([1, 1], f32, tag=f"st{j}_{l}")
                nc.vector.memset(lo[:], -16.0)
                nc.vector.memset(st[:], 32.0 / NPROBE)
                kk = float(KS[l])  # per-graph keep count
                for it in range(NITER):
                    pr_ = wp.tile([1, NPROBE], f32, tag="pr_")
                    nc.vector.tensor_scalar(out=pr_[:], in0=probei[:],
                                            scalar1=st[:, 0:1], scalar2=None,
                                            op0=OP.mult)
                    nc.vector.tensor_scalar(out=pr_[:], in0=pr_[:],
                                            scalar1=lo[:, 0:1], scalar2=None,
                                            op0=OP.add)
                    pb = pp.tile([128, NPROBE], f32, space="PSUM", tag="psB")
                    nc.tensor.matmul(pb[:], lhsT=ones_r[:], rhs=pr_[:],
                                     start=True, stop=True)
                    cmp_ = wp.tile([128, NPROBE, W2], f32, tag="cmp_")
                    nc.vector.tensor_tensor(
                        out=cmp_[:],
                        in0=sm_[:].rearrange("p (o w) -> p o w", o=1)
                            .to_broadcast([128, NPROBE, W2]),
                        in1=pb[:].rearrange("p (r o) -> p r o", o=1)
                            .to_broadcast([128, NPROBE, W2]),
                        op=OP.is_ge)
                    cnt = wp.tile([128, NPROBE], f32, tag="cnt")
                    nc.vector.tensor_reduce(out=cnt[:], in_=cmp_[:],
                                            axis=AX.X, op=OP.add)
                    cs = pp.tile([1, NPROBE], f32, space="PSUM", tag="psB")
                    nc.tensor.matmul(cs[:], lhsT=ones_c[:], rhs=cnt[:],
                                     start=True, stop=True)
                    sges = wp.tile([1, NPROBE], f32, tag="sges")
                    nc.vector.tensor_scalar(out=sges[:], in0=cs[:],
                                            scalar1=kk - 0.5, scalar2=None,
                                            op0=OP.is_ge)
                    s8 = wp.tile([1, 1], f32, tag="s8")
                    nc.vector.tensor_reduce(out=s8[:], in_=sges[:],
                                            axis=AX.X, op=OP.add)
                    nc.vector.tensor_tensor(out=s8[:], in0=s8[:],
                                            in1=st[:], op=OP.mult)
                    nc.vector.tensor_tensor(out=lo[:], in0=lo[:], in1=s8[:],
                                            op=OP.add)
                    nc.vector.tensor_scalar(out=st[:], in0=st[:],
                                            scalar1=1.0 / NPROBE,
                                            scalar2=None, op0=OP.mult)
                thr = wp.tile([1, 1], f32, tag="thr")
                nc.vector.tensor_scalar(out=thr[:], in0=st[:],
                                        scalar1=float(NPROBE) / 2,
                                        scalar2=None, op0=OP.mult)
                nc.vector.tensor_tensor(out=thr[:], in0=thr[:], in1=lo[:],
                                        op=OP.add)
                thb = pe_bcast_col(thr)
                nc.vector.tensor_scalar(out=na_g[:], in0=sm_[:],
                                        scalar1=thb[:, 0:1], scalar2=None,
                                        op0=OP.is_ge)
                if dbg and j == 0 and l == 0:
                    nc.gpsimd.dma_start(dbg['d_na0'].ap(), na_g[:])
                # f = na * tanh(score)
                nc.scalar.activation(f_g[:], score[:], AF.Tanh)
                nc.vector.tensor_tensor(out=f_g[:], in0=f_g[:], in1=na_g[:],
                                        op=OP.mult)
                # ---- mask xgT in place: xm = x * f  (column scale).
                # At l==2 nothing downstream needs xmT densely (readout
                # folds f into the attention weights), so skip the scale.
                if l < 2:
                    row_from_grid(f_g, frow, W2)
                    for t in range(NP // 512):
                        sl = slice(t * 512, (t + 1) * 512)
                        ob = pp.tile([128, 512], f32, space="PSUM", tag="psA")
                        nc.tensor.matmul(ob[:], lhsT=ones_rb[:],
                                         rhs=frow[0:1, sl], start=True,
                                         stop=True)
                        nc.vector.tensor_tensor(out=xgT[:, sl],
                                                in0=xgT[:, sl],
                                                in1=ob[:], op=OP.mult)
                    # rows of xm for the next layer's gather
                    transpose_pass(xgT, xrows2)
                # ---- readout (gx grid is pre-mask -> fold f)
                gxg = wp.tile([128, W2], f32, tag="gxg")
                nc.vector.tensor_tensor(out=gxg[:], in0=gxg_raw[:],
                                        in1=f_g[:], op=OP.mult)
                ex = wp.tile([128, W2], f32, tag="ex")
                nc.scalar.activation(ex[:], gxg[:], AF.Exp)
                nc.vector.tensor_tensor(out=ex[:], in0=ex[:], in1=na_g[:],
                                        op=OP.mult)
                smr = wp.tile([128, 1], f32, tag="smr")
                nc.vector.tensor_reduce(out=smr[:], in_=ex[:], axis=AX.X,
                                        op=OP.add)
                tot = pp.tile([1, 1], f32, space="PSUM", tag="psB")
                nc.tensor.matmul(tot[:], lhsT=smr[:], rhs=ones_c[:],
                                 start=True, stop=True)
                rden = wp.tile([1, 1], f32, tag="rden")
                nc.vector.reciprocal(rden[:], tot[:])
                rdb = pe_bcast_col(rden)
                ab = wp.tile([128, W2], bf16, tag="ab")
                nc.vector.tensor_scalar(out=ab[:], in0=ex[:],
                                        scalar1=rdb[:, 0:1], scalar2=None,
                                        op0=OP.mult)
                if l == 2:
                    nc.vector.tensor_tensor(out=ab[:], in0=ab[:],
                                            in1=f_g[:], op=OP.mult)
                row_from_grid(ab, abrow, W2)
                weighted_readout(xgT, abrow, ro2s[:, l, j:j + 1])

        # ====================================================== final MLP
        rob_in = dp.tile([GPC, 3 * 128], f32)
        rob_out = dp.tile([P, 3 * 128], f32)
        for j in range(GPC):
            nc.sync.dma_start(
                rob_in[j:j + 1, :].rearrange("g (l p) -> p (g l)", p=128),
                ro2s[:, :, j])
        nc.gpsimd.collective_compute(
            "AllGather", OP.bypass, replica_groups=grp,
            ins=[rob_in.opt()], outs=[rob_out.opt()])
        roall = cp.tile([32, 384], f32)
        nc.sync.dma_start(roall[:], rob_out[:])
        roT = cp.tile([128, 3, 33], f32)
        nc.vector.tensor_copy(out=roT[:, :, 0:1],
                              in_=ro1s[:].rearrange("p (l o) -> p l o", o=1))
        for lblk in range(3):
            tp = pp.tile([128, 32], f32, space="PSUM", tag="psB")
            nc.tensor.transpose(tp[:, 0:32],
                                roall[:, lblk * 128:(lblk + 1) * 128],
                                identf[0:32, 0:32])
            nc.vector.tensor_copy(out=roT[:, lblk, 1:33], in_=tp[:, 0:32])
        zp = pp.tile([33, 1], f32, space="PSUM", tag="psB")
        for lblk in range(3):
            nc.tensor.matmul(zp[:], lhsT=roT[:, lblk, :],
                             rhs=linw[:, lblk:lblk + 1], start=(lblk == 0),
                             stop=(lblk == 2))
        z = cp.tile([33, 1], f32)
        nc.scalar.activation(z[:], zp[:], AF.Tanh, bias=linb)
        h1p = pp.tile([48, 1], f32, space="PSUM", tag="psB")
        nc.tensor.matmul(h1p[:], lhsT=m1w, rhs=z[:], start=True, stop=True)
        h1 = cp.tile([48, 1], f32)
        nc.scalar.activation(h1[:], h1p[:], AF.Tanh, bias=m1b)
        h2p = pp.tile([16, 1], f32, space="PSUM", tag="psB")
        nc.tensor.matmul(h2p[:], lhsT=m2w, rhs=h1[:], start=True,
                         stop=True)
        h2 = cp.tile([16, 1], f32)
        nc.scalar.activation(h2[:], h2p[:], AF.Tanh, bias=m2b)
        h3p = pp.tile([1, 1], f32, space="PSUM", tag="psB")
        nc.tensor.matmul(h3p[:], lhsT=m3w, rhs=h2[:], start=True,
                         stop=True)
        h3 = cp.tile([1, 1], f32)
        nc.scalar.activation(h3[:], h3p[:], AF.Sigmoid, bias=m3b)
        nc.scalar.activation(h3[:], h3[:], AF.Sigmoid)
        nc.sync.dma_start(out.ap(), h3[:])
        if dbg:
            nc.gpsimd.dma_start(dbg['d_ro'].ap()[1:33, :], roall[:])
            nc.gpsimd.dma_start(
                dbg['d_ro'].ap()[0:1, :].rearrange("o (l p) -> p (o l)",
                                                   p=128), ro1s[:])

        for pool in (s2g, s2pool, dp, pp, wp, big, cp):
            pool.release()

    nc.compile()
    return nc


# ------------------------------------------------------------------- driver
def kernel(**inputs):
    per_core, meta = host_prep(inputs)
    key = meta
    if key not in _build_cache:
        _build_cache[key] = build_nc(*meta, debug=bool(
            int(__import__('os').environ.get('DMOI_DEBUG', '0'))))
    nc = _build_cache[key]
    import os as _os
    want_trace = bool(int(_os.environ.get('DMOI_TRACE', '0')))
    try:
        res = bass_utils.run_bass_kernel_spmd(
            nc, per_core, core_ids=list(range(NC)), trace=want_trace)
    except Exception:
        if not want_trace:
            raise
        res = bass_utils.run_bass_kernel_spmd(
            nc, per_core, core_ids=list(range(NC)))
    r0 = res.results[0]
    kernel.last_results = res
    return r0['out'].astype(np.float32)


# revision 8
# speedup vs baseline: 8.1136x; 1.6528x over previous
"""DeepMOI GNN kernel for 8 Trainium2 NeuronCores (Bass/Tile).

Sharding: stage-1 full-graph SAGE aggregation is sharded by dst-node slice
(1024 nodes/core) with an AllGather of the aggregated means; the dense
per-node compute is replicated. Stage-2 pathway subgraphs are data-parallel:
4 graphs per core, processed sequentially. readout2 rows are AllGathered
and the tiny final MLP is computed on every core.

Aggregation strategy: edges sorted by dst, packed into 128-dst windows;
segment-sum is a one-hot matmul on PE (one-hot built in one broadcast DVE
compare from static dst-local columns). Messages are bulk-gathered from a
row table in DRAM with dma_gather; the row tables are produced with XBAR
DMA transposes (no PE/scalar involvement). Global-attention readouts are
computed row-wise (gate.x via feature-contraction matmuls, then a
broadcast-weighted reduction) so no node-major transpose of x is needed.
Top-k masks use a 16-probe multisection search.

Host->device traffic is minimized: per-core-identical constants (weights,
h) ship sharded 1/8th per core and are AllGathered on device; gather
indices ship unreplicated ([16,X] int16, fanned out to 128 partitions on
device); reciprocal degrees are computed on device from the one-hot tiles;
iota/identity/window masks are generated on device. The jax persistent
compilation cache is enabled so repeat launches skip XLA re-compilation.
"""
import sys
sys.path.insert(0, '/opt/trn_rl_repo')
import numpy as np
import ml_dtypes

import jax
try:
    jax.config.update("jax_compilation_cache_dir", "/tmp/.dmoi_jax_cache")
    jax.config.update("jax_persistent_cache_min_entry_size_bytes", -1)
    jax.config.update("jax_persistent_cache_min_compile_time_secs", 0)
except Exception:
    pass

import concourse.bass as bass
import concourse.bacc as bacc
import concourse.tile as tile
import concourse.mybir as mybir
from concourse import bass_utils

N = 8000
NP = 8192
P = 32
ES = 4096
DIN = 16
D = 128
NC = 8
GPC = P // NC
W2 = 64
S1SLICE = NP // NC
S1W = S1SLICE // 128
KS = (6400, 5120, 4096)
NPROBE = 16
NITER = 4

AF = mybir.ActivationFunctionType
OP = mybir.AluOpType
dt = mybir.dt
BF = ml_dtypes.bfloat16
AX = mybir.AxisListType

_build_cache = {}

# Fb blob column map (f32 [128, FBW])
FB_BLA = 0          # [128, 1]
FB_BLS = 1          # [128, 5]
FB_BREL = 6         # [128, 3]
FB_NW = 9
FB_NB = 10
FB_NMS = 11
FB_LINW = 12        # [128, 3]
FB_M1W = 15         # [33, 48]
FB_M1B = 63         # [48, 1]
FB_M2W = 64         # [48, 16]
FB_M2B = 80         # [16, 1]
FB_M3W = 81         # [16, 1]
FB_M3B = 82         # [1, 1]
FB_LINB = 83        # [33, 1]
FBW = 84
# Wb blob column map (bf16 [128, WBW])
WB_WLS = 0          # [128, 5*128]
WB_WRS = 640        # [128, 5*128]
WB_GATE = 1280      # [128, 6]
WB_WREL = 1286      # [128, 3]
WB_WROOT = 1289     # [128, 3]
WBW = 1292


# ----------------------------------------------------------------- host prep
def wrap16(idx):
    n = idx.shape[0]
    return np.ascontiguousarray(idx.reshape(n // 16, 16).T.astype(np.int16))


def window_major_edges(src, dst, nwin, cmax, sentinel):
    order = np.argsort(dst, kind='stable')
    src, dst = src[order], dst[order]
    nslot = nwin * cmax * 128
    sp = np.full(nslot, sentinel, np.int64)
    dl = np.full((128, nwin * cmax), -1.0, np.float32)
    for w in range(nwin):
        lo = np.searchsorted(dst, w * 128)
        hi = np.searchsorted(dst, (w + 1) * 128)
        if hi <= lo:
            continue
        assert hi - lo <= cmax * 128, f"window {w}: {hi-lo}"
        base = w * cmax * 128
        sp[base:base + (hi - lo)] = src[lo:hi]
        for k in range(cmax):
            a = lo + k * 128
            if a >= hi:
                break
            b = min(hi, a + 128)
            dl[:b - a, w * cmax + k] = (dst[a:b] - w * 128).astype(np.float32)
    return sp, dl


def host_prep(inputs):
    h = np.asarray(inputs['h'], np.float32)
    ei = np.asarray(inputs['edge_index'], np.int64)
    sei = np.asarray(inputs['sub_edge_index'], np.int64)

    src1 = np.concatenate([ei[0], np.arange(N)])
    dst1 = np.concatenate([ei[1], np.arange(N)])
    cnt_w = np.bincount(dst1 // 128, minlength=64)
    CMAX1 = int(np.ceil(cnt_w.max() / 128))

    s1 = []
    for c in range(NC):
        m = (dst1 >= c * S1SLICE) & (dst1 < (c + 1) * S1SLICE)
        sp, dl = window_major_edges(src1[m], dst1[m] - c * S1SLICE,
                                    S1W, CMAX1, NP - 1)
        s1.append((wrap16(sp), dl.astype(BF)))

    hT = np.zeros((16, NP), BF)
    hT[:, :N] = h.T.astype(BF)

    ssrc = (sei[0].reshape(P, ES) - (np.arange(P) * N)[:, None])
    sdst = (sei[1].reshape(P, ES) - (np.arange(P) * N)[:, None])
    CMAX2 = 1
    for g in range(P):
        CMAX2 = max(CMAX2, int(np.ceil(
            np.bincount(sdst[g] // 128, minlength=W2).max() / 128)))

    s2src, s2dl = [], []
    for c in range(NC):
        a_s, a_d = [], []
        for j in range(GPC):
            g = c * GPC + j
            sp, dl = window_major_edges(ssrc[g], sdst[g], W2, CMAX2, NP - 1)
            a_s.append(wrap16(sp))
            a_d.append(dl)
        s2src.append(np.ascontiguousarray(np.concatenate(a_s, axis=1)))
        s2dl.append(np.ascontiguousarray(
            np.concatenate(a_d, axis=1).astype(BF)))

    W = {k: np.asarray(inputs[k], np.float32) for k in inputs}

    Fb = np.zeros((128, FBW), np.float32)
    Fb[:, FB_BLA] = W['bl_a']
    Fb[:, FB_BLS:FB_BLS + 5] = W['bl_s'].T
    Fb[:, FB_BREL:FB_BREL + 3] = np.tile(W['pool_brel'][None, :], (128, 1))
    Fb[:, FB_NW] = W['norm_w']
    Fb[:, FB_NB] = W['norm_b']
    Fb[:, FB_NMS] = W['norm_ms']
    Fb[:, FB_LINW:FB_LINW + 3] = W['lin_w'].reshape(3, 128).T
    Fb[0:33, FB_M1W:FB_M1W + 48] = W['m1_w']
    Fb[0:48, FB_M1B] = W['m1_b']
    Fb[0:48, FB_M2W:FB_M2W + 16] = W['m2_w']
    Fb[0:16, FB_M2B] = W['m2_b']
    Fb[0:16, FB_M3W] = W['m3_w'][:, 0]
    Fb[0, FB_M3B] = W['m3_b'][0]
    Fb[0:33, FB_LINB] = W['lin_b'][0]

    Wb = np.zeros((128, WBW), BF)
    Wb[:, WB_WLS:WB_WLS + 640] = np.ascontiguousarray(
        W['Wl_s'].transpose(1, 0, 2)).reshape(128, 640).astype(BF)
    Wb[:, WB_WRS:WB_WRS + 640] = np.ascontiguousarray(
        W['Wr_s'].transpose(1, 0, 2)).reshape(128, 640).astype(BF)
    Wb[:, WB_GATE:WB_GATE + 6] = W['gate_w'].T.astype(BF)
    Wb[:, WB_WREL:WB_WREL + 3] = W['pool_wrel'].T.astype(BF)
    Wb[:, WB_WROOT:WB_WROOT + 3] = W['pool_wroot'].T.astype(BF)

    WA = np.zeros((16, 256), BF)
    WA[:, 0:128] = W['Wl_a'].astype(BF)
    WA[:, 128:256] = W['Wr_a'].astype(BF)

    per_core = []
    for c in range(NC):
        m = dict(
            Wb_sh=np.ascontiguousarray(Wb[16 * c:16 * (c + 1), :]),
            Fb_sh=np.ascontiguousarray(Fb[16 * c:16 * (c + 1), :]),
            hT_sh=np.ascontiguousarray(hT[2 * c:2 * (c + 1), :]),
            WA_sh=np.ascontiguousarray(WA[2 * c:2 * (c + 1), :]),
        )
        m['s1_src'], m['s1_dl'] = s1[c]
        m['s2_src'], m['s2_dl'] = s2src[c], s2dl[c]
        per_core.append(m)
    return per_core, (CMAX1, CMAX2)


# ---------------------------------------------------------------- the kernel
def build_nc(CMAX1, CMAX2, debug=False):
    NP1 = S1W * CMAX1 * 128
    NP2 = W2 * CMAX2 * 128
    PR1 = S1W * CMAX1
    PR2 = W2 * CMAX2

    nc = bacc.Bacc("TRN2", target_bir_lowering=False, debug=False,
                   num_devices=NC)
    f32, bf16 = dt.float32, dt.bfloat16

    def inp(name, shape, d=f32):
        return nc.dram_tensor(name, shape, d, kind="ExternalInput")

    t_hT = inp('hT_sh', [2, NP], bf16)
    t_s1src = inp('s1_src', [16, NP1 // 16], dt.int16)
    t_s1dl = inp('s1_dl', [128, PR1], bf16)
    t_s2src = inp('s2_src', [16, GPC * NP2 // 16], dt.int16)
    t_s2dl = inp('s2_dl', [128, GPC * PR2], bf16)
    t_Wb = inp('Wb_sh', [16, WBW], bf16)
    t_WA = inp('WA_sh', [2, 256], bf16)
    t_Fb = inp('Fb_sh', [16, FBW])

    out = nc.dram_tensor('out', [1, 1], f32, kind="ExternalOutput")
    dbg = {}
    if debug:
        for nm, shp in (('d_x1T', [128, NP]), ('d_ro', [33, 384]),
                        ('d_sc0', [128, W2]), ('d_na0', [128, W2])):
            dbg[nm] = nc.dram_tensor(nm, shp, f32, kind="ExternalOutput")

    with tile.TileContext(nc, trace_sim=False) as tc:
        cp = tc.alloc_tile_pool(name="const", bufs=1)
        big = tc.alloc_tile_pool(name="big", bufs=1)
        wp = tc.alloc_tile_pool(name="wk", bufs=2)
        pp = tc.alloc_tile_pool(name="ps", bufs=2, space="PSUM")
        dp = tc.alloc_tile_pool(name="dram", bufs=1, space="DRAM")
        grp = [list(range(NC))]

        # ------------- packed constants (sharded upload + device AllGather).
        # Collectives cannot read IO tensors; bounce shards via internal DRAM.
        def gather_shared(t_sh, shard_shape, d):
            stage = dp.tile(shard_shape, d)
            nc.sync.dma_start(stage[:], t_sh.ap())
            full = dp.tile([shard_shape[0] * NC, shard_shape[1]], d)
            nc.gpsimd.collective_compute("AllGather", OP.bypass,
                                         replica_groups=grp,
                                         ins=[stage.opt()],
                                         outs=[full.opt()])
            return full

        Wb_full = gather_shared(t_Wb, [16, WBW], bf16)
        Wb = cp.tile([128, WBW], bf16, tag="Wb")
        nc.sync.dma_start(Wb[:], Wb_full[:])
        Fb_full = gather_shared(t_Fb, [16, FBW], f32)
        Fb = cp.tile([128, FBW], f32, tag="Fb")
        nc.sync.dma_start(Fb[:], Fb_full[:])
        WA_full = gather_shared(t_WA, [2, 256], bf16)
        WA = cp.tile([16, 256], bf16, tag="WA")
        nc.sync.dma_start(WA[:], WA_full[:])
        hT_full = gather_shared(t_hT, [2, NP], bf16)

        def WlS(i):
            return Wb[:, WB_WLS + i * 128:WB_WLS + (i + 1) * 128]

        def WrS(i):
            return Wb[:, WB_WRS + i * 128:WB_WRS + (i + 1) * 128]

        gate = Wb[:, WB_GATE:WB_GATE + 6]
        wrel = Wb[:, WB_WREL:WB_WREL + 3]
        wroot = Wb[:, WB_WROOT:WB_WROOT + 3]
        WlA = WA[:, 0:128]
        WrA = WA[:, 128:256]
        blA = Fb[:, FB_BLA:FB_BLA + 1]
        blS = Fb[:, FB_BLS:FB_BLS + 5]
        brel = Fb[:, FB_BREL:FB_BREL + 3]
        nw = Fb[:, FB_NW:FB_NW + 1]
        nb = Fb[:, FB_NB:FB_NB + 1]
        nms = Fb[:, FB_NMS:FB_NMS + 1]
        linw = Fb[:, FB_LINW:FB_LINW + 3]
        m1w = Fb[0:33, FB_M1W:FB_M1W + 48]
        m1b = Fb[0:48, FB_M1B:FB_M1B + 1]
        m2w = Fb[0:48, FB_M2W:FB_M2W + 16]
        m2b = Fb[0:16, FB_M2B:FB_M2B + 1]
        m3w = Fb[0:16, FB_M3W:FB_M3W + 1]
        m3b = Fb[0:1, FB_M3B:FB_M3B + 1]
        linb = Fb[0:33, FB_LINB:FB_LINB + 1]

        # ------------- generated constants
        iota = cp.tile([128, 128], f32)
        nc.gpsimd.iota(iota[:], pattern=[[1, 128]], base=0,
                       channel_multiplier=0,
                       allow_small_or_imprecise_dtypes=True)
        ones128 = cp.tile([128, 128], f32)
        nc.vector.memset(ones128[:], 1.0)
        identf = cp.tile([128, 128], f32)
        nc.gpsimd.affine_select(identf[:], ones128[:], pattern=[[-1, 128]],
                                compare_op=OP.is_equal, fill=0.0, base=0,
                                channel_multiplier=1)
        nid = cp.tile([128, W2], f32)
        nc.gpsimd.iota(nid[:], pattern=[[128, W2]], base=0,
                       channel_multiplier=1,
                       allow_small_or_imprecise_dtypes=True)
        na0c = cp.tile([128, W2], f32)
        nc.vector.tensor_scalar(out=na0c[:], in0=nid[:],
                                scalar1=float(N) - 0.5, scalar2=None,
                                op0=OP.is_ge)
        nc.vector.tensor_scalar(out=na0c[:], in0=na0c[:], scalar1=-1.0,
                                scalar2=1.0, op0=OP.mult, op1=OP.add)
        probei = cp.tile([1, NPROBE], f32)
        nc.gpsimd.iota(probei[:], pattern=[[1, NPROBE]], base=1,
                       channel_multiplier=0,
                       allow_small_or_imprecise_dtypes=True)
        ones_c = cp.tile([128, 1], f32); nc.vector.memset(ones_c[:], 1.0)
        ones_cb = cp.tile([128, 1], bf16); nc.vector.memset(ones_cb[:], 1.0)
        ones_r = cp.tile([1, 128], f32); nc.vector.memset(ones_r[:], 1.0)
        ones_rb = cp.tile([1, 128], bf16); nc.vector.memset(ones_rb[:], 1.0)

        xT = big.tile([128, NP], bf16, tag="xT")       # current features^T
        rows = big.tile([128, W2, 128], bf16, tag="rows")
        ro2s = cp.tile([128, 3, GPC], f32)
        ro1s = cp.tile([128, 3], f32)
        xrows1 = dp.tile([NP, 128], bf16)              # stage-1 row table
        xrows2 = dp.tile([NP, 128], bf16)              # stage-2 scratch table
        utab = dp.tile([NP, 128], bf16)                # u table (DRAM)
        xTd = dp.tile([128, NP], bf16)                 # transpose staging

        # =========================================================== helpers
        def transpose_pass(srcT, dst):
            """srcT [128, NP] bf16 -> row table dst [NP, 128] via XBAR DMA
            transposes (bounced through DRAM; no PE/scalar involvement)."""
            nc.sync.dma_start(xTd[:], srcT[:])
            for w in range(W2):
                nc.sync.dma_start_transpose(rows[:, w, :],
                                            xTd[:, w * 128:(w + 1) * 128])
            nc.sync.dma_start(
                dst[:].rearrange("(w p) e -> p w e", p=128), rows[:])

        rowbounce = dp.tile([1, NP], bf16)
        frow = cp.tile([1, NP], bf16)
        abrow = cp.tile([1, NP], bf16)

        def row_from_grid(gtile, rowt, wn):
            nc.gpsimd.dma_start(
                rowbounce[0:1, 0:wn * 128].rearrange("o (w p) -> o p w",
                                                     p=128), gtile[:])
            nc.sync.dma_start(rowt[0:1, 0:wn * 128],
                              rowbounce[0:1, 0:wn * 128])

        # valid-node row mask [1, NP] (1 for node < N)
        narow = cp.tile([1, NP], bf16)
        row_from_grid(na0c, narow, W2)

        def pe_bcast_col(val11):
            """[1,1] f32 -> psum [128,1]."""
            ps = pp.tile([128, 1], f32, space="PSUM", tag="psB")
            nc.tensor.matmul(ps[:], lhsT=ones_r[:], rhs=val11[:],
                             start=True, stop=True)
            return ps

        def weighted_readout(xsrcT, ab_row, slot):
            """slot[d] = sum_n ab_row[n] * xsrcT[d, n]."""
            parts = wp.tile([128, NP // 512], f32, tag="parts")
            for t in range(NP // 512):
                sl = slice(t * 512, (t + 1) * 512)
                ob = pp.tile([128, 512], f32, space="PSUM", tag="psA")
                nc.tensor.matmul(ob[:], lhsT=ones_rb[:],
                                 rhs=ab_row[0:1, sl], start=True, stop=True)
                tmp = wp.tile([128, 512], f32, tag="tmpws")
                nc.vector.tensor_tensor(out=tmp[:], in0=ob[:],
                                        in1=xsrcT[:, sl], op=OP.mult)
                nc.vector.tensor_reduce(out=parts[:, t:t + 1], in_=tmp[:],
                                        axis=AX.X, op=OP.add)
            nc.vector.tensor_reduce(out=slot, in_=parts[:], axis=AX.X,
                                    op=OP.add)

        def readout1(layer):
            """stage-1 global attention over xT, row-wise."""
            sums = wp.tile([1, NP // 512], f32, tag="sums")
            for t in range(NP // 512):
                sl = slice(t * 512, (t + 1) * 512)
                gp = pp.tile([1, 512], f32, space="PSUM", tag="psB")
                nc.tensor.matmul(gp[:], lhsT=gate[:, layer:layer + 1],
                                 rhs=xT[:, sl], start=True, stop=True)
                exr = wp.tile([1, 512], f32, tag="exr")
                nc.scalar.activation(exr[:], gp[:], AF.Exp)
                nc.vector.tensor_tensor(out=abrow[0:1, sl], in0=exr[:],
                                        in1=narow[0:1, sl], op=OP.mult)
                nc.vector.tensor_reduce(out=sums[:, t:t + 1],
                                        in_=abrow[0:1, sl], axis=AX.X,
                                        op=OP.add)
            tot = wp.tile([1, 1], f32, tag="tot1")
            nc.vector.tensor_reduce(out=tot[:], in_=sums[:], axis=AX.X,
                                    op=OP.add)
            rden = wp.tile([1, 1], f32, tag="rden")
            nc.vector.reciprocal(rden[:], tot[:])
            nc.vector.tensor_scalar(out=abrow[:], in0=abrow[:],
                                    scalar1=rden[:, 0:1], scalar2=None,
                                    op0=OP.mult)
            weighted_readout(xT, abrow, ro1s[:, layer:layer + 1])

        # ====================================================== stage 1
        s1pool = tc.alloc_tile_pool(name="s1", bufs=1)
        s1src = s1pool.tile([128, NP1 // 16], dt.int16, tag="s1src")
        for k in range(8):
            nc.sync.dma_start(s1src[16 * k:16 * (k + 1), :], t_s1src.ap())
        s1dlb = s1pool.tile([128, PR1], bf16, tag="s1dlb")
        nc.sync.dma_start(s1dlb[:], t_s1dl.ap())
        s1dl = s1pool.tile([128, PR1], f32, tag="s1dl")
        nc.vector.tensor_copy(out=s1dl[:], in_=s1dlb[:])
        eq1 = s1pool.tile([128, PR1, 128], bf16)
        nc.vector.tensor_tensor(
            out=eq1[:],
            in0=iota[:].rearrange("p (o e) -> p o e", o=1)
                .to_broadcast([128, PR1, 128]),
            in1=s1dl[:].rearrange("p (c o) -> p c o", o=1)
                .to_broadcast([128, PR1, 128]),
            op=OP.is_equal)

        # reciprocal in-degree of this core's dst slice (same for all layers)
        degp1 = pp.tile([128, S1W], f32, space="PSUM", tag="psB")
        for w in range(S1W):
            for k in range(CMAX1):
                pr = w * CMAX1 + k
                nc.tensor.matmul(degp1[:, w:w + 1], lhsT=eq1[:, pr, :],
                                 rhs=ones_cb[:], start=(k == 0),
                                 stop=(k == CMAX1 - 1))
        rdeg1g = wp.tile([128, S1W], f32, tag="rdeg1g")
        nc.vector.tensor_scalar(out=rdeg1g[:], in0=degp1[:], scalar1=1.0,
                                scalar2=None, op0=OP.max)
        nc.vector.reciprocal(rdeg1g[:], rdeg1g[:])
        row_from_grid(rdeg1g, frow, S1W)
        rdegb = s1pool.tile([128, S1SLICE], bf16, tag="rdegb")
        for t in range(S1SLICE // 512):
            sl = slice(t * 512, (t + 1) * 512)
            ob = pp.tile([128, 512], f32, space="PSUM", tag="psA")
            nc.tensor.matmul(ob[:], lhsT=ones_rb[:], rhs=frow[0:1, sl],
                             start=True, stop=True)
            nc.scalar.copy(out=rdegb[:, sl], in_=ob[:])

        # publish h rows (feature-padded with zeros) for the layer-0 gather
        nc.vector.memset(xT[:], 0.0)
        nc.sync.dma_start(xT[0:16, :], hT_full[:])
        transpose_pass(xT, xrows1)

        agb_in = dp.tile([128, S1SLICE], bf16)
        agb_out = dp.tile([128 * NC, S1SLICE], bf16)
        meanT = big.tile([128, NP], bf16, tag="meanT")

        def s1_layer(layer):
            srcT = xT
            aggp = pp.tile([128, S1W, 128], f32, space="PSUM", tag="pagg")
            for half in range(2):
                msgs = s1pool.tile([128, PR1 // 2, 128], bf16, tag="msgs1")
                nc.gpsimd.dma_gather(
                    out_ap=msgs[:], in_ap=xrows1[:],
                    idxs_ap=s1src[:, half * (NP1 // 32):(half + 1) * (NP1 // 32)],
                    num_idxs=NP1 // 2, num_idxs_reg=NP1 // 2, elem_size=128,
                    single_packet=False)
                for w in range(S1W // 2 * half, S1W // 2 * (half + 1)):
                    for k in range(CMAX1):
                        pr = w * CMAX1 + k
                        lpr = pr - half * (PR1 // 2)
                        nc.tensor.matmul(aggp[:, w, :], lhsT=msgs[:, lpr, :],
                                         rhs=eq1[:, pr, :], start=(k == 0),
                                         stop=(k == CMAX1 - 1))
            mslice = wp.tile([128, S1SLICE], bf16, tag="mslice")
            nc.vector.tensor_tensor(
                out=mslice[:], in0=aggp[:].rearrange("p w e -> p (w e)"),
                in1=rdegb[:], op=OP.mult)
            nc.sync.dma_start(agb_in[:], mslice[:])
            nc.gpsimd.collective_compute(
                "AllGather", OP.bypass, replica_groups=grp,
                ins=[agb_in.opt()], outs=[agb_out.opt()])
            # reassemble meanT full [128, NP]
            nc.sync.dma_start(
                meanT[:].rearrange("p (c s) -> p c s", c=NC),
                agb_out[:].rearrange("(c p) s -> p c s", p=128))
            Wl = WlA if layer == 0 else WlS(layer - 1)
            Wr = WrA if layer == 0 else WrS(layer - 1)
            bl = blA if layer == 0 else blS[:, layer - 1:layer]
            kdim = 16 if layer == 0 else 128
            for t in range(NP // 512):
                sl = slice(t * 512, (t + 1) * 512)
                xp = pp.tile([128, 512], f32, space="PSUM", tag="psA")
                nc.tensor.matmul(xp[:], lhsT=Wl[0:kdim, :],
                                 rhs=meanT[0:kdim, sl], start=True, stop=False)
                nc.tensor.matmul(xp[:], lhsT=Wr[0:kdim, :],
                                 rhs=srcT[0:kdim, sl], start=False, stop=True)
                nc.scalar.activation(xT[:, sl], xp[:], AF.Tanh, bias=bl)

        def gnorm():
            mu = wp.tile([128, 1], f32, tag="mu")
            nc.vector.tensor_reduce(out=mu[:], in_=xT[:, 0:N], axis=AX.X,
                                    op=OP.add)
            nc.vector.tensor_scalar(out=mu[:], in0=mu[:], scalar1=1.0 / N,
                                    scalar2=None, op0=OP.mult)
            mums = wp.tile([128, 1], f32, tag="mums")
            nc.vector.tensor_tensor(out=mums[:], in0=mu[:], in1=nms[:],
                                    op=OP.mult)
            o = meanT  # meanT is free between layers; reuse as scratch
            nc.vector.tensor_scalar(out=o[:], in0=xT[:], scalar1=mums[:],
                                    scalar2=None, op0=OP.subtract)
            var = wp.tile([128, 1], f32, tag="var")
            sq = rows[:].rearrange("p w e -> p (w e)")  # rows free here too
            nc.vector.tensor_tensor(out=sq, in0=o[:], in1=o[:], op=OP.mult)
            nc.vector.tensor_reduce(out=var[:], in_=sq[:, 0:N], axis=AX.X,
                                    op=OP.add)
            nc.vector.tensor_scalar(out=var[:], in0=var[:], scalar1=1.0 / N,
                                    scalar2=1e-5, op0=OP.mult, op1=OP.add)
            rstd = wp.tile([128, 1], f32, tag="rstd")
            nc.vector.reciprocal(rstd[:], var[:])
            nc.scalar.activation(rstd[:], rstd[:], AF.Sqrt)
            sc = wp.tile([128, 1], f32, tag="scn")
            nc.vector.tensor_tensor(out=sc[:], in0=rstd[:], in1=nw[:],
                                    op=OP.mult)
            nc.vector.tensor_scalar(out=xT[:], in0=o[:], scalar1=sc[:],
                                    scalar2=nb[:], op0=OP.mult, op1=OP.add)

        for layer in range(3):
            s1_layer(layer)
            readout1(layer)
            if layer < 2:
                gnorm()
            transpose_pass(xT, xrows1)

        if dbg:
            nc.gpsimd.dma_start(dbg['d_x1T'].ap(), xT[:])

        s1pool.release()

        # ====================================================== stage 2
        s2pool = tc.alloc_tile_pool(name="s2", bufs=1)
        s2g = tc.alloc_tile_pool(name="s2g", bufs=1)
        s2src = s2pool.tile([128, GPC * NP2 // 16], dt.int16, tag="s2src")
        for k in range(8):
            nc.sync.dma_start(s2src[16 * k:16 * (k + 1), :], t_s2src.ap())
        s2dlb = s2pool.tile([128, GPC * PR2], bf16, tag="s2dlb")
        nc.sync.dma_start(s2dlb[:], t_s2dl.ap())
        s2dl = s2pool.tile([128, GPC * PR2], f32, tag="s2dl")
        nc.vector.tensor_copy(out=s2dl[:], in_=s2dlb[:])

        xgT = s2pool.tile([128, NP], bf16, tag="xgT")
        eq2 = s2pool.tile([128, PR2, 128], bf16, tag="eq2")
        rbc = s2pool.tile([128, NP], bf16, tag="rbc")
        ubounce = dp.tile([3, NP], bf16)

        def ugT(sl):
            # u/uroot/gx rows live in partitions 0:3 of the `rows` staging
            # tile — temporally disjoint from its transpose-staging use.
            return rows[:].rearrange("p w e -> p (w e)")[0:3, sl]
        ones_pr = s2pool.tile([128, PR2], bf16, tag="ones_pr")
        nc.vector.memset(ones_pr[:], 1.0)

        for j in range(GPC):
            # per-graph eq tiles (unscaled one-hot), one broadcast compare
            nc.vector.tensor_tensor(
                out=eq2[:],
                in0=iota[:].rearrange("p (o e) -> p o e", o=1)
                    .to_broadcast([128, PR2, 128]),
                in1=s2dl[:, j * PR2:(j + 1) * PR2]
                    .rearrange("p (c o) -> p c o", o=1)
                    .to_broadcast([128, PR2, 128]),
                op=OP.is_equal)
            nc.vector.tensor_copy(out=xgT[:], in_=xT[:])
            na_g = cp.tile([128, W2], f32, tag=f"nag{j}")
            f_g = cp.tile([128, W2], f32, tag=f"fg{j}")
            nc.vector.tensor_copy(out=na_g[:], in_=na0c[:])

            for l in range(3):
                li = 2 + l
                # ---- messages gather (stage-1 table at l=0, else own)
                msgs = s2g.tile([128, PR2, 128], bf16, tag="m2")
                nc.gpsimd.dma_gather(
                    out_ap=msgs[:], in_ap=(xrows1[:] if l == 0 else xrows2[:]),
                    idxs_ap=s2src[:, j * (NP2 // 16):(j + 1) * (NP2 // 16)],
                    num_idxs=NP2, num_idxs_reg=NP2, elem_size=128,
                    single_packet=False)
                # ---- reciprocal alive-in-degree -> rbc [128, NP]
                if l == 0:
                    ind = ones_pr          # all edges live before 1st pool
                else:
                    ind = wp.tile([128, PR2], bf16, tag="ind")
                    nc.vector.tensor_scalar(out=ind[:],
                                            in0=msgs[:, :, 0:1].rearrange(
                                                "p c o -> p (c o)"),
                                            scalar1=0.0, scalar2=None,
                                            op0=OP.not_equal)
                degp = pp.tile([128, W2], f32, space="PSUM", tag="psB")
                for w in range(W2):
                    for k in range(CMAX2):
                        pr = w * CMAX2 + k
                        nc.tensor.matmul(
                            degp[:, w:w + 1], lhsT=eq2[:, pr, :],
                            rhs=ind[:, pr:pr + 1], start=(k == 0),
                            stop=(k == CMAX2 - 1))
                rdeg = wp.tile([128, W2], f32, tag="rdeg")
                nc.vector.tensor_scalar(out=rdeg[:], in0=degp[:],
                                        scalar1=1.0, scalar2=None,
                                        op0=OP.max)
                nc.vector.reciprocal(rdeg[:], rdeg[:])
                nc.vector.tensor_tensor(out=rdeg[:], in0=rdeg[:],
                                        in1=na_g[:], op=OP.mult)
                row_from_grid(rdeg, frow, W2)
                for t in range(NP // 512):
                    sl = slice(t * 512, (t + 1) * 512)
                    ob = pp.tile([128, 512], f32, space="PSUM", tag="psA")
                    nc.tensor.matmul(ob[:], lhsT=ones_rb[:],
                                     rhs=frow[0:1, sl], start=True,
                                     stop=True)
                    nc.scalar.copy(out=rbc[:, sl], in_=ob[:])
                # ---- aggregation matmuls + fused mean scale
                for grpi in range(8):
                    agp = pp.tile([128, 8, 128], f32, space="PSUM",
                                  tag="pagg")
                    for wi in range(8):
                        w = grpi * 8 + wi
                        for k in range(CMAX2):
                            pr = w * CMAX2 + k
                            nc.tensor.matmul(agp[:, wi, :],
                                             lhsT=msgs[:, pr, :],
                                             rhs=eq2[:, pr, :],
                                             start=(k == 0),
                                             stop=(k == CMAX2 - 1))
                    sl = slice(grpi * 1024, (grpi + 1) * 1024)
                    nc.vector.tensor_tensor(
                        out=meanT[:, sl],
                        in0=agp[:].rearrange("p w e -> p (w e)"),
                        in1=rbc[:, sl], op=OP.mult)
                # ---- x' = tanh(Wl.T meanT + Wr.T xm + bl)
                for t in range(NP // 512):
                    sl = slice(t * 512, (t + 1) * 512)
                    xp = pp.tile([128, 512], f32, space="PSUM", tag="psA")
                    nc.tensor.matmul(xp[:], lhsT=WlS(li), rhs=meanT[:, sl],
                                     start=True, stop=False)
                    nc.tensor.matmul(xp[:], lhsT=WrS(li), rhs=xgT[:, sl],
                                     start=False, stop=True)
                    nc.scalar.activation(xgT[:, sl], xp[:], AF.Tanh,
                                         bias=blS[:, li:li + 1])
                # ---- u/uroot/gx rows via feature-contraction matmuls
                wcols = cp.tile([128, 3], bf16, tag=f"wcols{j}_{l}")
                nc.vector.tensor_copy(out=wcols[:, 0:1], in_=wrel[:, l:l + 1])
                nc.vector.tensor_copy(out=wcols[:, 1:2],
                                      in_=wroot[:, l:l + 1])
                nc.vector.tensor_copy(out=wcols[:, 2:3],
                                      in_=gate[:, 3 + l:4 + l])
                for t in range(NP // 512):
                    sl = slice(t * 512, (t + 1) * 512)
                    up = pp.tile([3, 512], f32, space="PSUM", tag="psB")
                    nc.tensor.matmul(up[:], lhsT=wcols[:], rhs=xgT[:, sl],
                                     start=True, stop=True)
                    nc.scalar.copy(out=ugT(sl), in_=up[:])
                # u table rows + uroot/gx grids (bounced through DRAM)
                nc.sync.dma_start(ubounce[:], ugT(slice(None)))
                nc.gpsimd.dma_start(utab[:, 0:1],
                                    ubounce[0:1, :].rearrange("o n -> n o"))
                urootg = wp.tile([128, W2], bf16, tag="urootg")
                nc.sync.dma_start(
                    urootg[:],
                    ubounce[1:2, :].rearrange("o (w p) -> p (o w)", p=128))
                gxg_raw = wp.tile([128, W2], bf16, tag="gxgr")
                nc.sync.dma_start(
                    gxg_raw[:],
                    ubounce[2:3, :].rearrange("o (w p) -> p (o w)", p=128))
                # ---- score pass
                umsg = s2g.tile([128, PR2, 128], bf16, tag="m2")
                nc.gpsimd.dma_gather(
                    out_ap=umsg[:], in_ap=utab[:],
                    idxs_ap=s2src[:, j * (NP2 // 16):(j + 1) * (NP2 // 16)],
                    num_idxs=NP2, num_idxs_reg=NP2, elem_size=128,
                    single_packet=False)
                scp = pp.tile([128, W2], f32, space="PSUM", tag="psB")
                for w in range(W2):
                    for k in range(CMAX2):
                        pr = w * CMAX2 + k
                        nc.tensor.matmul(scp[:, w:w + 1], lhsT=eq2[:, pr, :],
                                         rhs=umsg[:, pr, 0:1],
                                         start=(k == 0),
                                         stop=(k == CMAX2 - 1))
                score = cp.tile([128, W2], f32, tag=f"score{j}_{l}")
                nc.vector.tensor_tensor(out=score[:], in0=scp[:],
                                        in1=urootg[:], op=OP.add)
                nc.vector.tensor_scalar(out=score[:], in0=score[:],
                                        scalar1=brel[:, l:l + 1],
                                        scalar2=None, op0=OP.add)
                if dbg and j == 0 and l == 0:
                    nc.gpsimd.dma_start(dbg['d_sc0'].ap(), score[:])
                # ---- top-k threshold via multiprobe
                sm_ = cp.tile([128, W2], f32, tag=f"smask{j}_{l}")
                nc.vector.tensor_tensor(out=sm_[:], in0=score[:], in1=na_g[:],
                                        op=OP.mult)
                pen = wp.tile([128, W2], f32, tag="pen")
                nc.vector.tensor_scalar(out=pen[:], in0=na_g[:],
                                        scalar1=1e6, scalar2=-1e6,
                                        op0=OP.mult, op1=OP.add)
                nc.vector.tensor_tensor(out=sm_[:], in0=sm_[:], in1=pen[:],
                                        op=OP.add)
                lo = cp.tile([1, 1], f32, tag=f"lo{j}_{l}")
                st = cp.tile([1, 1], f32, tag=f"st{j}_{l}")
                nc.vector.memset(lo[:], -16.0)
                nc.vector.memset(st[:], 32.0 / NPROBE)
                kk = float(KS[l])  # per-graph keep count
                for it in range(NITER):
                    pr_ = wp.tile([1, NPROBE], f32, tag="pr_")
                    nc.vector.tensor_scalar(out=pr_[:], in0=probei[:],
                                            scalar1=st[:, 0:1], scalar2=None,
                                            op0=OP.mult)
                    nc.vector.tensor_scalar(out=pr_[:], in0=pr_[:],
                                            scalar1=lo[:, 0:1], scalar2=None,
                                            op0=OP.add)
                    pb = pp.tile([128, NPROBE], f32, space="PSUM", tag="psB")
                    nc.tensor.matmul(pb[:], lhsT=ones_r[:], rhs=pr_[:],
                                     start=True, stop=True)
                    cmp_ = wp.tile([128, NPROBE, W2], f32, tag="cmp_")
                    nc.vector.tensor_tensor(
                        out=cmp_[:],
                        in0=sm_[:].rearrange("p (o w) -> p o w", o=1)
                            .to_broadcast([128, NPROBE, W2]),
                        in1=pb[:].rearrange("p (r o) -> p r o", o=1)
                            .to_broadcast([128, NPROBE, W2]),
                        op=OP.is_ge)
                    cnt = wp.tile([128, NPROBE], f32, tag="cnt")
                    nc.vector.tensor_reduce(out=cnt[:], in_=cmp_[:],
                                            axis=AX.X, op=OP.add)
                    cs = pp.tile([1, NPROBE], f32, space="PSUM", tag="psB")
                    nc.tensor.matmul(cs[:], lhsT=ones_c[:], rhs=cnt[:],
                                     start=True, stop=True)
                    sges = wp.tile([1, NPROBE], f32, tag="sges")
                    nc.vector.tensor_scalar(out=sges[:], in0=cs[:],
                                            scalar1=kk - 0.5, scalar2=None,
                                            op0=OP.is_ge)
                    s8 = wp.tile([1, 1], f32, tag="s8")
                    nc.vector.tensor_reduce(out=s8[:], in_=sges[:],
                                            axis=AX.X, op=OP.add)
                    nc.vector.tensor_tensor(out=s8[:], in0=s8[:],
                                            in1=st[:], op=OP.mult)
                    nc.vector.tensor_tensor(out=lo[:], in0=lo[:], in1=s8[:],
                                            op=OP.add)
                    nc.vector.tensor_scalar(out=st[:], in0=st[:],
                                            scalar1=1.0 / NPROBE,
                                            scalar2=None, op0=OP.mult)
                thr = wp.tile([1, 1], f32, tag="thr")
                nc.vector.tensor_scalar(out=thr[:], in0=st[:],
                                        scalar1=float(NPROBE) / 2,
                                        scalar2=None, op0=OP.mult)
                nc.vector.tensor_tensor(out=thr[:], in0=thr[:], in1=lo[:],
                                        op=OP.add)
                thb = pe_bcast_col(thr)
                nc.vector.tensor_scalar(out=na_g[:], in0=sm_[:],
                                        scalar1=thb[:, 0:1], scalar2=None,
                                        op0=OP.is_ge)
                if dbg and j == 0 and l == 0:
                    nc.gpsimd.dma_start(dbg['d_na0'].ap(), na_g[:])
                # f = na * tanh(score)
                nc.scalar.activation(f_g[:], score[:], AF.Tanh)
                nc.vector.tensor_tensor(out=f_g[:], in0=f_g[:], in1=na_g[:],
                                        op=OP.mult)
                # ---- mask xgT in place: xm = x * f  (column scale).
                # At l==2 nothing downstream needs xmT densely (readout
                # folds f into the attention weights), so skip the scale.
                if l < 2:
                    row_from_grid(f_g, frow, W2)
                    for t in range(NP // 512):
                        sl = slice(t * 512, (t + 1) * 512)
                        ob = pp.tile([128, 512], f32, space="PSUM", tag="psA")
                        nc.tensor.matmul(ob[:], lhsT=ones_rb[:],
                                         rhs=frow[0:1, sl], start=True,
                                         stop=True)
                        nc.vector.tensor_tensor(out=xgT[:, sl],
                                                in0=xgT[:, sl],
                                                in1=ob[:], op=OP.mult)
                    # rows of xm for the next layer's gather
                    transpose_pass(xgT, xrows2)
                # ---- readout (gx grid is pre-mask -> fold f)
                gxg = wp.tile([128, W2], f32, tag="gxg")
                nc.vector.tensor_tensor(out=gxg[:], in0=gxg_raw[:],
                                        in1=f_g[:], op=OP.mult)
                ex = wp.tile([128, W2], f32, tag="ex")
                nc.scalar.activation(ex[:], gxg[:], AF.Exp)
                nc.vector.tensor_tensor(out=ex[:], in0=ex[:], in1=na_g[:],
                                        op=OP.mult)
                smr = wp.tile([128, 1], f32, tag="smr")
                nc.vector.tensor_reduce(out=smr[:], in_=ex[:], axis=AX.X,
                                        op=OP.add)
                tot = pp.tile([1, 1], f32, space="PSUM", tag="psB")
                nc.tensor.matmul(tot[:], lhsT=smr[:], rhs=ones_c[:],
                                 start=True, stop=True)
                rden = wp.tile([1, 1], f32, tag="rden")
                nc.vector.reciprocal(rden[:], tot[:])
                rdb = pe_bcast_col(rden)
                ab = wp.tile([128, W2], bf16, tag="ab")
                nc.vector.tensor_scalar(out=ab[:], in0=ex[:],
                                        scalar1=rdb[:, 0:1], scalar2=None,
                                        op0=OP.mult)
                if l == 2:
                    nc.vector.tensor_tensor(out=ab[:], in0=ab[:],
                                            in1=f_g[:], op=OP.mult)
                row_from_grid(ab, abrow, W2)
                weighted_readout(xgT, abrow, ro2s[:, l, j:j + 1])

        # ====================================================== final MLP
        rob_in = dp.tile([GPC, 3 * 128], f32)
        rob_out = dp.tile([P, 3 * 128], f32)
        for j in range(GPC):
            nc.sync.dma_start(
                rob_in[j:j + 1, :].rearrange("g (l p) -> p (g l)", p=128),
                ro2s[:, :, j])
        nc.gpsimd.collective_compute(
            "AllGather", OP.bypass, replica_groups=grp,
            ins=[rob_in.opt()], outs=[rob_out.opt()])
        roall = cp.tile([32, 384], f32)
        nc.sync.dma_start(roall[:], rob_out[:])
        roT = cp.tile([128, 3, 33], f32)
        nc.vector.tensor_copy(out=roT[:, :, 0:1],
                              in_=ro1s[:].rearrange("p (l o) -> p l o", o=1))
        for lblk in range(3):
            tp = pp.tile([128, 32], f32, space="PSUM", tag="psB")
            nc.tensor.transpose(tp[:, 0:32],
                                roall[:, lblk * 128:(lblk + 1) * 128],
                                identf[0:32, 0:32])
            nc.vector.tensor_copy(out=roT[:, lblk, 1:33], in_=tp[:, 0:32])
        zp = pp.tile([33, 1], f32, space="PSUM", tag="psB")
        for lblk in range(3):
            nc.tensor.matmul(zp[:], lhsT=roT[:, lblk, :],
                             rhs=linw[:, lblk:lblk + 1], start=(lblk == 0),
                             stop=(lblk == 2))
        z = cp.tile([33, 1], f32)
        nc.scalar.activation(z[:], zp[:], AF.Tanh, bias=linb)
        h1p = pp.tile([48, 1], f32, space="PSUM", tag="psB")
        nc.tensor.matmul(h1p[:], lhsT=m1w, rhs=z[:], start=True, stop=True)
        h1 = cp.tile([48, 1], f32)
        nc.scalar.activation(h1[:], h1p[:], AF.Tanh, bias=m1b)
        h2p = pp.tile([16, 1], f32, space="PSUM", tag="psB")
        nc.tensor.matmul(h2p[:], lhsT=m2w, rhs=h1[:], start=True,
                         stop=True)
        h2 = cp.tile([16, 1], f32)
        nc.scalar.activation(h2[:], h2p[:], AF.Tanh, bias=m2b)
        h3p = pp.tile([1, 1], f32, space="PSUM", tag="psB")
        nc.tensor.matmul(h3p[:], lhsT=m3w, rhs=h2[:], start=True,
                         stop=True)
        h3 = cp.tile([1, 1], f32)
        nc.scalar.activation(h3[:], h3p[:], AF.Sigmoid, bias=m3b)
        nc.scalar.activation(h3[:], h3[:], AF.Sigmoid)
        nc.sync.dma_start(out.ap(), h3[:])
        if dbg:
            nc.gpsimd.dma_start(dbg['d_ro'].ap()[1:33, :], roall[:])
            nc.gpsimd.dma_start(
                dbg['d_ro'].ap()[0:1, :].rearrange("o (l p) -> p (o l)",
                                                   p=128), ro1s[:])

        for pool in (s2g, s2pool, dp, pp, wp, big, cp):
            pool.release()

    nc.compile()
    return nc


# ------------------------------------------------------------------- driver
def kernel(**inputs):
    per_core, meta = host_prep(inputs)
    key = meta
    if key not in _build_cache:
        _build_cache[key] = build_nc(*meta, debug=bool(
            int(__import__('os').environ.get('DMOI_DEBUG', '0'))))
    nc = _build_cache[key]
    if not hasattr(nc, '_dmoi_json_cache'):
        # The module is immutable after compile; memoize its serialization
        # (re-lowered on every run_bass_kernel_spmd call otherwise).
        nc._dmoi_json_cache = nc.to_json_bytes()
        nc.to_json_bytes = lambda: nc._dmoi_json_cache
    import os as _os
    want_trace = bool(int(_os.environ.get('DMOI_TRACE', '0')))
    try:
        res = bass_utils.run_bass_kernel_spmd(
            nc, per_core, core_ids=list(range(NC)), trace=want_trace)
    except Exception:
        if not want_trace:
            raise
        res = bass_utils.run_bass_kernel_spmd(
            nc, per_core, core_ids=list(range(NC)))
    r0 = res.results[0]
    kernel.last_results = res
    return r0['out'].astype(np.float32)
